# revision 1
# baseline (speedup 1.0000x reference)
"""CGNN message-passing kernel for Trainium2, 8 NeuronCores.

Strategy:
  - Algebraic reduction: the edge attention gate depends only on the SOURCE
    node, so attention collapses to a per-node scalar alpha. The whole edge
    computation becomes  aggr = dinv ⊙ (A @ (dinv ⊙ (alpha*xn + (1-alpha)*xa)))
    with A the (multi)adjacency + self loops.
  - Node phase (data-parallel over 8 cores): each core computes its shard of
    the per-node message table m' = dinv*(alpha*xn + (1-alpha)*xa); AllGather
    replicates the full [NP, 128] table to every core.
  - Edge phase (target-sharded): edges sorted by (target tile, table half)
    on host; tiles dealt to cores balanced by chunk count. Per 128-target
    tile: bulk dma_gather segments (<=1024 int16 indices each, 4 SWDGE
    queues) pull source rows from the bf16 table; a single broadcast-AP
    is_equal builds all one-hot chunks from (col % 128); scatter-add becomes
    PSUM-accumulated matmuls; update/classifier layers fuse feature-major.

Host work is limited to index preprocessing (sort/bincount/layout) and
shard/unshard data movement; all FLOPs run on device.
"""
import numpy as np

N_CORES = 8
P = 128
IN_DIM = 256
HID = 128
HALF = 64
OUT_DIM = 2
LRELU_SLOPE = 0.01
SLAB_TILES = 4          # node-phase tiles per slab (nn <= 512)


def _host_plan(x, edge_index):
    """Index preprocessing + data layout. Returns dict of np arrays + meta."""
    n = x.shape[0]
    NP = ((n + 1023) // 1024) * 1024          # divisible by 8*128
    nsh = NP // N_CORES                        # nodes per core
    t_c = nsh // P                             # target tiles per core
    ntiles = NP // P

    ei = np.asarray(edge_index)
    row = ei[0].astype(np.int64)
    col = ei[1].astype(np.int64)
    loops = np.arange(n, dtype=np.int64)
    row_a = np.concatenate([row, loops])
    col_a = np.concatenate([col, loops])

    deg = np.bincount(col_a, minlength=NP).astype(np.float32)
    deg[n:] = 1.0                              # pad nodes: keep m' finite

    order = np.argsort(col_a, kind="stable")
    rs = row_a[order].astype(np.int32)
    cs = col_a[order]

    h0 = NP // 2
    assert h0 <= 32767, "table half exceeds int16 index range"
    # order edges by (tile, half) so each tile's lo-half edges precede hi-half
    half_e = (rs >= h0).astype(np.int64)
    key = (cs // P) * 2 + half_e
    order2 = np.argsort(key, kind="stable")
    rs = rs[order2]
    cs = cs[order2]
    key = key[order2]

    gb = np.searchsorted(key, np.arange(0, 2 * (NP // P) + 1))  # group bounds
    glo = gb[0:-1:2]
    ghi = gb[1::2]
    gend = gb[2::2]
    n_lo = ghi - glo                           # per tile lo-edge counts
    n_hi = gend - ghi
    kl_j = -(-n_lo // P)                       # lo chunks per tile
    kh_j = -(-n_hi // P)
    t_slots = ntiles // N_CORES                # == t_c
    tile_order = np.lexsort((kh_j, kl_j))      # tiles sorted by (kl, kh)
    # assign[c, s] = global tile handled by core c at slot s
    assign = tile_order.reshape(t_slots, N_CORES).T
    kl_a = kl_j[assign]                        # [cores, slots]
    kh_a = kh_j[assign]
    KL = np.maximum(1, kl_a.max(0))
    KH = np.maximum(1, kh_a.max(0))
    kmax_tot = int((KL + KH).max())
    kmax8 = int(max(KL.max(), KH.max())) * 8

    # local int16 indices + colmod per (tile, group-chunk-slot)
    idx_lo = np.zeros((ntiles, P, int(KL.max())), np.int16)
    idx_hi = np.zeros((ntiles, P, int(KH.max())), np.int16)
    cm_all = np.full((ntiles, P, kmax_tot), 999.0, np.float32)
    m = len(cs)
    j_e = (cs // P).astype(np.int64)
    is_hi = rs >= h0
    epos = np.arange(m, dtype=np.int64)
    epos = np.where(is_hi, epos - ghi[j_e], epos - glo[j_e])
    c_e = epos // P
    p_e = epos % P
    lo_m = ~is_hi
    idx_lo[j_e[lo_m], p_e[lo_m], c_e[lo_m]] = rs[lo_m].astype(np.int16)
    idx_hi[j_e[is_hi], p_e[is_hi], c_e[is_hi]] = (rs[is_hi] - h0).astype(np.int16)
    cm_e = (cs - j_e * P).astype(np.float32)
    slot_of = np.empty(ntiles, np.int64)
    slot_of[tile_order] = np.arange(ntiles) // N_CORES
    c_cm = np.where(is_hi, KL[slot_of[j_e]] + c_e, c_e)
    cm_all[j_e, p_e, c_cm] = cm_e

    # wrap + replicate indices for the 8 gpsimd cores: [P, K*8] int16 where
    # block [16g:16g+16, c*8:(c+1)*8] holds chunk c's idxs transposed-wrapped
    def wrap_rep(arr):      # [ntiles, P, K] -> [ntiles, P, K*8]
        nt, _, k = arr.shape
        flat = arr.transpose(0, 2, 1).reshape(nt, k * P)       # chunk-major
        blk = flat.reshape(nt, k * 8, 16).transpose(0, 2, 1)   # [nt, 16, k*8]
        out = np.repeat(blk, 8, axis=0).reshape(nt, 8 * 16, k * 8)
        return np.ascontiguousarray(out)

    idx_cat = np.zeros((ntiles, P, kmax8 * 2), np.int16)
    wlo = wrap_rep(idx_lo)
    whi = wrap_rep(idx_hi)
    idx_cat[:, :, :wlo.shape[2]] = wlo
    idx_cat[:, :, kmax8:kmax8 + whi.shape[2]] = whi

    x_t = np.zeros((IN_DIM, NP), np.float32)
    x_t[:, :n] = np.asarray(x, np.float32).T

    iota = np.tile(np.arange(P, dtype=np.float32), (P, 1))

    deg_ct = np.stack([deg[c * nsh:(c + 1) * nsh].reshape(t_c, P).T
                       for c in range(N_CORES)])   # [cores, 128, T_C]
    # deg for the edge-phase tiles in assignment order: [cores, T_C*P]
    deg_tiles = deg.reshape(ntiles, P)
    deg_et = deg_tiles[assign].reshape(N_CORES, t_c * P)
    return dict(NP=NP, NSH=nsh, T_C=t_c, H0=h0, KL=KL, KH=KH, deg_ct=deg_ct,
                deg_et=deg_et, assign=assign,
                KMAX8=kmax8, KMAX_TOT=kmax_tot,
                idx_all=idx_cat, cm_all=cm_all, deg=deg, x_t=x_t, iota=iota)


def _build_program(meta, with_collective=True, act_lrelu=True, edge_bf16=True,
                   repeat=1, ablate=()):
    import concourse.bass as bass
    import concourse.bacc as bacc
    import concourse.mybir as mybir
    import concourse.tile as tile
    from concourse.masks import make_identity

    f32 = mybir.dt.float32
    bf16 = mybir.dt.bfloat16
    edt = bf16 if edge_bf16 else f32
    i16 = mybir.dt.int16
    NSH, T_C, NP, H0 = meta["NSH"], meta["T_C"], meta["NP"], meta["H0"]
    KL, KH = meta["KL"], meta["KH"]
    KMAX8, KMAX_TOT = meta["KMAX8"], meta["KMAX_TOT"]
    AF = mybir.ActivationFunctionType
    Alu = mybir.AluOpType

    nc = bacc.Bacc("TRN2", target_bir_lowering=False, debug=False,
               num_swdge_queues=4)
    table = nc.dram_tensor("cc_table", [NP, HID],
                           bf16 if edge_bf16 else f32,
                           addr_space="Shared")

    def emit_lrelu(out_ap, psum_ap, bias_ap, tmp):
        # out = leaky_relu(psum + bias); ACT Lrelu on HW, decomposition in sim
        if act_lrelu:
            nc.scalar.activation(out_ap, psum_ap, AF.Lrelu, bias=bias_ap,
                                 alpha=LRELU_SLOPE)
        else:
            nc.scalar.activation(out_ap, psum_ap, AF.Identity, bias=bias_ap)
            nc.vector.tensor_scalar(out=tmp, in0=out_ap, scalar1=LRELU_SLOPE,
                                    scalar2=None, op0=Alu.mult)
            nc.vector.tensor_tensor(out=out_ap, in0=out_ap, in1=tmp,
                                    op=Alu.max)

    # ---- external inputs (per-core shards unless noted)
    d_xt = nc.dram_tensor("x_t", [IN_DIM, NSH], f32, kind="ExternalInput")
    d_deg = nc.dram_tensor("deg", [NSH], f32, kind="ExternalInput")
    d_degct = nc.dram_tensor("deg_ct", [P, T_C], f32, kind="ExternalInput")
    d_deget = nc.dram_tensor("deg_et", [T_C * P], f32, kind="ExternalInput")
    d_idx = nc.dram_tensor("idx", [T_C, P, KMAX8 * 2], i16, kind="ExternalInput")
    d_cm = nc.dram_tensor("cm", [T_C, P, KMAX_TOT], f32, kind="ExternalInput")
    d_iota = nc.dram_tensor("iota", [P, P], f32, kind="ExternalInput")
    d_w_in = nc.dram_tensor("W_in", [IN_DIM, HID], f32, kind="ExternalInput")
    d_b_in = nc.dram_tensor("b_in", [HID, 1], f32, kind="ExternalInput")
    d_w_nor = nc.dram_tensor("W_nor", [HALF, HID], f32, kind="ExternalInput")
    d_b_nor = nc.dram_tensor("b_nor", [HID, 1], f32, kind="ExternalInput")
    d_w_ab = nc.dram_tensor("W_abnor", [HALF, HID], f32, kind="ExternalInput")
    d_b_ab = nc.dram_tensor("b_abnor", [HID, 1], f32, kind="ExternalInput")
    d_w_att = nc.dram_tensor("W_att", [HID, HID], f32, kind="ExternalInput")
    d_b_att = nc.dram_tensor("b_att", [HID, 1], f32, kind="ExternalInput")
    d_v_att = nc.dram_tensor("v_att", [HID, 1], f32, kind="ExternalInput")
    d_w_upd = nc.dram_tensor("W_upd", [HID, HID], f32, kind="ExternalInput")
    d_b_upd = nc.dram_tensor("b_upd", [HID, 1], f32, kind="ExternalInput")
    d_w_cls = nc.dram_tensor("W_cls", [HID, OUT_DIM], f32, kind="ExternalInput")
    d_b_cls = nc.dram_tensor("b_cls", [OUT_DIM, 1], f32, kind="ExternalInput")
    d_out = nc.dram_tensor("outp", [OUT_DIM, NSH], f32, kind="ExternalOutput")

    with tile.TileContext(nc) as tc:
        with (
            tc.tile_pool(name="const", bufs=1) as cpool,
            tc.tile_pool(name="sbuf", bufs=3) as pool,
            tc.tile_pool(name="sb3", bufs=3) as pool3,
            tc.tile_pool(name="dram", bufs=1, space="DRAM") as dpool,
        ):
            rep_cm = tc.For_i(0, repeat, 1) if repeat > 1 else None
            if rep_cm is not None:
                rep_cm.__enter__()
            # ---------- persistent constants ----------
            w_in_a = cpool.tile([P, HID], f32)
            w_in_b = cpool.tile([P, HID], f32)
            nc.sync.dma_start(w_in_a[:], d_w_in[:P, :])
            nc.sync.dma_start(w_in_b[:], d_w_in[P:, :])
            w_nor = cpool.tile([P, HID], f32)     # zero-extended K=128
            w_ab = cpool.tile([P, HID], f32)
            nc.vector.memset(w_nor[:], 0.0)
            nc.vector.memset(w_ab[:], 0.0)
            nc.sync.dma_start(w_nor[:HALF, :], d_w_nor[:])
            nc.sync.dma_start(w_ab[HALF:, :], d_w_ab[:])
            w_att = cpool.tile([P, HID], f32)
            nc.sync.dma_start(w_att[:], d_w_att[:])
            v_att = cpool.tile([P, 1], f32)
            nc.sync.dma_start(v_att[:], d_v_att[:])
            w_upd = cpool.tile([P, HID], f32)
            nc.sync.dma_start(w_upd[:], d_w_upd[:])
            w_cls = cpool.tile([P, OUT_DIM], f32)
            nc.sync.dma_start(w_cls[:], d_w_cls[:])
            b_in = cpool.tile([P, 1], f32)
            nc.sync.dma_start(b_in[:], d_b_in[:])
            b_nor = cpool.tile([P, 1], f32)
            nc.sync.dma_start(b_nor[:], d_b_nor[:])
            b_ab = cpool.tile([P, 1], f32)
            nc.sync.dma_start(b_ab[:], d_b_ab[:])
            b_att = cpool.tile([P, 1], f32)
            nc.sync.dma_start(b_att[:], d_b_att[:])
            b_upd = cpool.tile([P, 1], f32)
            nc.sync.dma_start(b_upd[:], d_b_upd[:])
            b_cls = cpool.tile([OUT_DIM, 1], f32)
            nc.sync.dma_start(b_cls[:], d_b_cls[:])
            iota_t = cpool.tile([P, P], f32)
            nc.sync.dma_start(iota_t[:], d_iota[:])
            ones_r = cpool.tile([1, P], f32)
            nc.vector.memset(ones_r[:], 1.0)
            ident = cpool.tile([P, P], f32)
            make_identity(nc, ident[:])

            # dinv columns [p, tile] for this core's nodes
            dct = cpool.tile([P, T_C], f32)
            nc.sync.dma_start(dct[:], d_degct[:])
            nc.scalar.activation(dct[:], dct[:], AF.Sqrt)
            nc.vector.reciprocal(dct[:], dct[:])

            # dinv row for this core's ASSIGNED edge tiles: 1/sqrt(deg)
            dinvr = cpool.tile([1, NSH], f32)
            nc.sync.dma_start(dinvr[:], d_deget[:][None, :])
            nc.scalar.activation(dinvr[:], dinvr[:], AF.Sqrt)
            nc.vector.reciprocal(dinvr[:], dinvr[:])

            # message table (gather source) + local shard
            shard = dpool.tile([NSH, HID], edt)

            # ---------- node phase (this core's NSH nodes) ----------
            npsum = tc.tile_pool(name="npsum", bufs=2, space="PSUM")
            pp1 = pp2 = npsum.__enter__()
            slabs = []
            t0 = 0
            while t0 < T_C:
                nt = min(SLAB_TILES, T_C - t0)
                slabs.append((t0, nt))
                t0 += nt
            for (s0, nt) in (slabs if 'node' not in ablate else []):
                nn = nt * P
                nb = s0 * P
                xta = pool.tile([P, 512], f32, tag="xta")
                xtb = pool.tile([P, 512], f32, tag="xtb")
                nc.sync.dma_start(xta[:, :nn], d_xt[:P, nb:nb + nn])
                nc.sync.dma_start(xtb[:, :nn], d_xt[P:, nb:nb + nn])
                ph = pp1.tile([P, 512], f32, tag="ph")
                nc.tensor.matmul(ph[:, :nn], w_in_a[:], xta[:, :nn],
                                 start=True, stop=False)
                nc.tensor.matmul(ph[:, :nn], w_in_b[:], xtb[:, :nn],
                                 start=False, stop=True)
                h = pool.tile([P, 512], f32, tag="h")
                ltmp = pool.tile([P, 512], f32, tag="ltmp")
                emit_lrelu(h[:, :nn], ph[:, :nn], b_in[:], ltmp[:, :nn])
                pn = pp1.tile([P, 512], f32, tag="pn")
                pa = pp1.tile([P, 512], f32, tag="pa")
                nc.tensor.matmul(pn[:, :nn], w_nor[:], h[:, :nn],
                                 start=True, stop=True)
                nc.tensor.matmul(pa[:, :nn], w_ab[:], h[:, :nn],
                                 start=True, stop=True)
                xn = pool.tile([P, 512], f32, tag="xn")
                xa = pool.tile([P, 512], f32, tag="xa")
                nc.scalar.activation(xn[:, :nn], pn[:, :nn], AF.Identity,
                                     bias=b_nor[:])
                nc.scalar.activation(xa[:, :nn], pa[:, :nn], AF.Identity,
                                     bias=b_ab[:])
                s_t = pool.tile([P, 512], f32, tag="s")
                nc.vector.tensor_add(s_t[:, :nn], xn[:, :nn], xa[:, :nn])
                patt = pp1.tile([P, 512], f32, tag="ph")
                nc.tensor.matmul(patt[:, :nn], w_att[:], s_t[:, :nn],
                                 start=True, stop=True)
                hatt = pool.tile([P, 512], f32, tag="hatt")
                nc.scalar.activation(hatt[:, :nn], patt[:, :nn], AF.Tanh,
                                     bias=b_att[:])
                for j in range(nt):
                    jn = j * P
                    jt = s0 + j
                    pal = pp2.tile([P, 1], f32, tag="tr")
                    nc.tensor.matmul(pal[:], hatt[:, jn:jn + P], v_att[:],
                                     start=True, stop=True)
                    sig = pool.tile([P, 1], f32, tag="sig")
                    nc.scalar.activation(sig[:], pal[:], AF.Sigmoid)
                    a2c = pool.tile([P, 1], f32, tag="a2c")
                    nc.vector.tensor_mul(a2c[:], sig[:], dct[:, jt:jt + 1])
                    dac = pool.tile([P, 1], f32, tag="dac")
                    nc.vector.tensor_sub(dac[:], dct[:, jt:jt + 1], a2c[:])
                    ptr1 = pp2.tile([P, P], f32, tag="tr")
                    nc.tensor.transpose(ptr1[:], xn[:, jn:jn + P], ident[:])
                    ptr2 = pp2.tile([P, P], f32, tag="tr")
                    nc.tensor.transpose(ptr2[:], xa[:, jn:jn + P], ident[:])
                    t2 = pool.tile([P, P], edt, tag="t2")
                    nc.vector.tensor_scalar(out=t2[:], in0=ptr1[:],
                                            scalar1=a2c[:, :1], scalar2=None,
                                            op0=Alu.mult)
                    t3 = pool.tile([P, P], edt, tag="t3")
                    nc.vector.tensor_scalar(out=t3[:], in0=ptr2[:],
                                            scalar1=dac[:, :1], scalar2=None,
                                            op0=Alu.mult)
                    mrow = pool.tile([P, P], edt, tag="mrow")
                    nc.vector.tensor_add(mrow[:], t2[:], t3[:])
                    nc.sync.dma_start(
                        shard[jt * P:(jt + 1) * P, :], mrow[:])

            npsum.__exit__(None, None, None)

            # ---------- replicate table ----------
            if with_collective:
                nc.gpsimd.collective_compute(
                    "AllGather",
                    mybir.AluOpType.bypass,
                    replica_groups=[list(range(N_CORES))],
                    ins=[shard.opt()],
                    outs=[table[:]],
                )
            else:
                # cost-model / single-core mode: fake it with a local copy
                nc.sync.dma_start(table[:NSH, :], shard[:])

            # ---------- edge phase (this core's T_C target tiles) ----------
            epsum = tc.tile_pool(name="epsum", bufs=3, space="PSUM")
            pp2 = epsum.__enter__()
            for j in range(T_C if 'edge' not in ablate else 0):
                kl, kh = int(KL[j]), int(KH[j])
                kt = kl + kh
                it = pool3.tile([P, KMAX8 * 2], i16, tag="it")
                nc.sync.dma_start(it[:], d_idx[j])
                cmt = pool3.tile([P, KMAX_TOT], f32, tag="cmt")
                nc.sync.dma_start(cmt[:], d_cm[j])
                g = pool3.tile([P, KMAX_TOT * HID], edt, tag="g")
                # SWDGE ring caps one gather at 1024 descriptors -> <=8 chunks;
                # split each group into equal-ish segments for queue balance
                segs = []
                for (ktot, dst0, base, io0) in ((kl, 0, 0, 0),
                                                (kh, kl, H0, KMAX8)):
                    nsp = -(-ktot // 8)
                    c0 = 0
                    for q in range(nsp):
                        nseg = (ktot - c0) // (nsp - q)
                        segs.append((dst0 + c0, nseg, base, io0 + c0 * 8))
                        c0 += nseg
                for si, (dst_c, nseg, base, io) in enumerate(segs if 'gather' not in ablate else []):
                    nc.gpsimd.dma_gather(
                        out_ap=g[:, dst_c * HID:(dst_c + nseg) * HID]
                        .rearrange("p (c f) -> p c f", f=HID),
                        in_ap=table[base:base + H0, :],
                        idxs_ap=it[:, io:io + nseg * 8],
                        num_idxs=nseg * P,
                        num_idxs_reg=nseg * P,
                        elem_size=HID,
                        queue_num=(j * 4 + si) % 4,
                    )
                s_oh = pool3.tile([P, KMAX_TOT * P], edt, tag="soh")
                if 'onehot' not in ablate:
                    nc.vector.tensor_tensor(
                    out=s_oh[:, :kt * P].rearrange("p (k f) -> p k f", f=P),
                    in0=iota_t[:].unsqueeze(1).to_broadcast([P, kt, P]),
                    in1=cmt[:, :kt].unsqueeze(2).to_broadcast([P, kt, P]),
                    op=Alu.is_equal)
                pagg = pp2.tile([P, P], f32, tag="pagg")
                if 'aggmm' in ablate:
                    nc.tensor.matmul(pagg[:], g[:, 0:HID], s_oh[:, 0:P],
                                     start=True, stop=True)
                for c in range(kt if 'aggmm' not in ablate else 0):
                    nc.tensor.matmul(pagg[:], g[:, c * HID:(c + 1) * HID],
                                     s_oh[:, c * P:(c + 1) * P],
                                     start=(c == 0), stop=(c == kt - 1))
                jn = j * P
                pdb = pp2.tile([P, P], f32, tag="emisc")
                nc.tensor.matmul(pdb[:], ones_r[:], dinvr[:, jn:jn + P],
                                 start=True, stop=True)
                dbc = pool.tile([P, P], f32, tag="dbc")
                nc.scalar.activation(dbc[:], pdb[:], AF.Identity)
                m2 = pool.tile([P, P], f32, tag="m2")
                nc.vector.tensor_mul(m2[:], pagg[:], dbc[:])
                pu = pp2.tile([P, P], f32, tag="emisc")
                nc.tensor.matmul(pu[:], w_upd[:], m2[:], start=True, stop=True)
                lu = pool.tile([P, P], f32, tag="lu")
                ltm2 = pool.tile([P, P], f32, tag="ltm2")
                emit_lrelu(lu[:], pu[:], b_upd[:], ltm2[:])
                po = pp2.tile([OUT_DIM, P], f32, tag="emisc")
                nc.tensor.matmul(po[:], w_cls[:], lu[:], start=True, stop=True)
                ot = pool.tile([OUT_DIM, P], f32, tag="ot")
                nc.scalar.activation(ot[:], po[:], AF.Identity, bias=b_cls[:])
                nc.sync.dma_start(d_out[:, jn:jn + P], ot[:])
            epsum.__exit__(None, None, None)
            if rep_cm is not None:
                rep_cm.__exit__(None, None, None)

    nc.compile()
    return nc


def _run_spmd_presharded(nc, in_maps, n_cores=N_CORES):
    """Run a compiled Bass program on n_cores via PJRT with host-side
    pre-sharded inputs (avoids XLA reshard programs on big arrays)."""
    import jax
    import concourse.mybir as mybir
    from concourse import bass2jax
    from jax.sharding import Mesh, PartitionSpec, NamedSharding
    from jax.experimental.shard_map import shard_map

    bass2jax.install_neuronx_cc_hook()
    partition_name = nc.partition_id_tensor.name if nc.partition_id_tensor else None
    in_names, out_names, out_avals, zero_outs = [], [], [], []
    for alloc in nc.m.functions[0].allocations:
        if not isinstance(alloc, mybir.MemoryLocationSet):
            continue
        name = alloc.memorylocations[0].name
        if alloc.kind == "ExternalInput":
            if name != partition_name:
                in_names.append(name)
        elif alloc.kind == "ExternalOutput":
            out_names.append(name)
            shape = tuple(alloc.tensor_shape)
            dtype = mybir.dt.np(alloc.dtype)
            out_avals.append(jax.core.ShapedArray(shape, dtype))
            zero_outs.append(np.zeros(shape, dtype))
    n_params = len(in_names)
    in_names_all = list(in_names) + out_names
    if partition_name is not None:
        in_names_all.append(partition_name)

    def _body(*args):
        operands = list(args)
        if partition_name is not None:
            operands.append(bass2jax.partition_id_tensor())
        outs = bass2jax._bass_exec_p.bind(
            *operands,
            out_avals=tuple(out_avals),
            in_names=tuple(in_names_all),
            out_names=tuple(out_names),
            lowering_input_output_aliases=(),
            sim_require_finite=True,
            sim_require_nnan=True,
            nc=nc,
        )
        return tuple(outs)

    devices = jax.devices()[:n_cores]
    mesh = Mesh(np.asarray(devices), ("core",))
    spec = PartitionSpec("core")
    n_outs = len(out_avals)
    sharded = jax.jit(
        shard_map(_body, mesh=mesh, in_specs=(spec,) * (n_params + n_outs),
                  out_specs=(spec,) * n_outs, check_rep=False),
        keep_unused=True,
    )
    sh = NamedSharding(mesh, spec)

    def put(per_core_arrays):
        a0 = np.asarray(per_core_arrays[0])
        gshape = (n_cores * a0.shape[0],) + a0.shape[1:]
        shards = [jax.device_put(np.ascontiguousarray(per_core_arrays[c]),
                                 devices[c]) for c in range(n_cores)]
        return jax.make_array_from_single_device_arrays(gshape, sh, shards)

    args = [put([m[name] for m in in_maps]) for name in in_names]
    args += [put([z] * n_cores) for z in zero_outs]
    out_arrs = sharded(*args)
    jax.block_until_ready(out_arrs)
    return [
        {name: np.asarray(out_arrs[i]).reshape(n_cores, *out_avals[i].shape)[c]
         for i, name in enumerate(out_names)}
        for c in range(n_cores)
    ]


def kernel(x, edge_index, W_in, b_in, W_nor, b_nor, W_abnor, b_abnor,
           W_att, b_att, v_att, W_upd, b_upd, W_cls, b_cls):
    x = np.asarray(x, np.float32)
    n = x.shape[0]
    meta = _host_plan(x, edge_index)
    NSH, T_C = meta["NSH"], meta["T_C"]
    nc = _build_program(meta, with_collective=True)

    shared = {
        "iota": meta["iota"],
        "W_in": np.asarray(W_in, np.float32),
        "b_in": np.asarray(b_in, np.float32).reshape(HID, 1),
        "W_nor": np.asarray(W_nor, np.float32),
        "b_nor": np.asarray(b_nor, np.float32).reshape(HID, 1),
        "W_abnor": np.asarray(W_abnor, np.float32),
        "b_abnor": np.asarray(b_abnor, np.float32).reshape(HID, 1),
        "W_att": np.asarray(W_att, np.float32),
        "b_att": np.asarray(b_att, np.float32).reshape(HID, 1),
        "v_att": np.asarray(v_att, np.float32).reshape(HID, 1),
        "W_upd": np.asarray(W_upd, np.float32),
        "b_upd": np.asarray(b_upd, np.float32).reshape(HID, 1),
        "W_cls": np.asarray(W_cls, np.float32),
        "b_cls": np.asarray(b_cls, np.float32).reshape(OUT_DIM, 1),
    }
    idx_c = meta["idx_all"][meta["assign"]]     # [cores, T_C, P, KMAX8*2]
    cm_c = meta["cm_all"][meta["assign"]]
    in_maps = []
    for c in range(N_CORES):
        in_maps.append({
            **shared,
            "x_t": np.ascontiguousarray(meta["x_t"][:, c * NSH:(c + 1) * NSH]),
            "deg": meta["deg"][c * NSH:(c + 1) * NSH],
            "deg_ct": meta["deg_ct"][c],
            "deg_et": meta["deg_et"][c],
            "idx": idx_c[c],
            "cm": cm_c[c],
        })

    results = _run_spmd_presharded(nc, in_maps)
    NPD = meta["NP"]
    out_full = np.empty((NPD, OUT_DIM), np.float32)
    assign = meta["assign"]
    for c in range(N_CORES):
        oc = results[c]["outp"].T.reshape(T_C, P, OUT_DIM)   # per slot
        for s_i in range(T_C):
            jt = assign[c, s_i]
            out_full[jt * P:(jt + 1) * P] = oc[s_i]
    return np.ascontiguousarray(out_full[:n])



# revision 22
# speedup vs baseline: 1.3628x; 1.3628x over previous
"""CGNN message-passing kernel for Trainium2, 8 NeuronCores.

Strategy (v3):
  - Algebraic reduction: the attention gate depends only on the source node,
    so the edge computation collapses to aggr = dinv (x) (A @ m') with
    m'[j] = dinv_j*(alpha_j*xn_j + (1-alpha_j)*xa_j) a per-node table.
  - Contiguous node sharding: core c owns nodes [c*NSH, (c+1)*NSH) for both
    phases; self-loop terms are added from SBUF (no gathering of self rows).
  - Node phase is fully feature-major and slab-batched: the alpha logit is a
    [1, n] row matmul, alpha*dinv and dinv broadcasts are outer-product
    matmuls, the message mix is three slab-wide DVE ops; only the final
    node-major table rows need a PE transpose per tile. Sigmoid is realized
    as 0.5*tanh(q/2)+0.5 (v_att pre-halved) so Lrelu/Tanh/Identity share one
    activation table.
  - Edge phase: edges sorted by (4-tile quad, table half, target); one SWDGE
    dma_gather per (quad, half) with exact max-count index streams. The
    scatter-add is PSUM-accumulated matmuls over a narrow one-hot: chunks of
    128 target-sorted edges span only ~8 targets, so each matmul writes a
    32-column window of the packed [128, 512] quad PSUM. Window offsets are
    compile-time (union over cores, computed per run); per-core variability
    lives in the cm data (sentinels mask pad slots). The one-hot is built in
    [p, t, k] layout against a materialized iota so all DVE operands are
    packed 2-byte -> 2x DVE rate.
"""
import numpy as np

N_CORES = 8
P = 128
IN_DIM = 256
HID = 128
HALF = 64
OUT_DIM = 2
LRELU_SLOPE = 0.01
SLAB = 4                  # node-phase tiles per slab (<=512 cols)
G = 4                     # edge-phase tiles per quad
W = 32                    # one-hot window width
SENT = 200.0              # one-hot sentinel (never matches a window col)


def _wrap16(flat):
    """[n] int16 -> [128, ceil(n/16)] in the SWDGE 16-wrap + 8x replicated
    layout (flat position i lands at row i%16, col i//16)."""
    n = len(flat)
    cols = -(-n // 16)
    pad = np.zeros(cols * 16, np.int16)
    pad[:n] = flat
    blk = pad.reshape(cols, 16).T          # [16, cols]
    return np.tile(blk, (8, 1))            # [128, cols]


def _host_plan(x, edge_index):
    n = x.shape[0]
    NP = ((n + 1023) // 1024) * 1024       # 50176
    NSH = NP // N_CORES                    # 6272
    T_C = NSH // P                         # 49
    H0 = NP // 2                           # 25088 (int16-safe table half)
    NQ = -(-T_C // G)                      # 13 quads (last has 1 tile)

    ei = np.asarray(edge_index)
    row = ei[0].astype(np.int64)
    col = ei[1].astype(np.int64)

    deg = np.bincount(col, minlength=NP).astype(np.float32)
    deg[:n] += 1.0                         # self loops
    deg[n:] = 1.0                          # pad nodes stay finite

    core_e = col // NSH
    half_e = (row >= H0).astype(np.int64)
    lt_e = (col % NSH) // P                # local tile 0..48
    q_e = lt_e // G

    # per (core, quad, half) edge counts -> static stream sizes
    cnt = np.zeros((N_CORES, NQ, 2), np.int64)
    np.add.at(cnt, (core_e, q_e, half_e), 1)
    nmax = cnt.max(axis=0)                 # [NQ, 2]
    nidx_stat = ((nmax + 127) // 128) * 128  # num_idxs per gather
    CQH = -(-nidx_stat // P)               # gather chunks
    idx_cols = nidx_stat // 16
    idx_off = np.zeros((NQ, 2), np.int64)
    io = 0
    for q in range(NQ):
        for h in range(2):
            idx_off[q, h] = io
            io += idx_cols[q, h]
    IDXC = int(io)
    CMAXG = int(CQH.max())

    order = np.lexsort((col, half_e, q_e, core_e))
    rs = row[order]
    cs = col[order]
    bounds = np.concatenate([[0], np.cumsum(cnt.reshape(-1))])

    # per-core streams: sources, tile-in-quad, local col
    streams = {}
    for c in range(N_CORES):
        for q in range(NQ):
            for h in range(2):
                bi = (c * NQ + q) * 2 + h
                b0, b1 = bounds[bi], bounds[bi + 1]
                streams[(c, q, h)] = (rs[b0:b1] - h * H0,
                                      (cs[b0:b1] % NSH) // P - q * G,
                                      (cs[b0:b1] % P))

    # jobs: per (q, h): [(chunk, tile-in-quad, window offset)], windows from
    # the union of target cols over cores (exact, computed per run)
    jobs = {}
    slot_off = np.zeros((NQ, 2), np.int64)
    so = 0
    KSLOT = 0
    for q in range(NQ):
        for h in range(2):
            slot_off[q, h] = so
            jl = []
            ucols = {}
            for c in range(N_CORES):
                _, tbs, lcs = streams[(c, q, h)]
                pos = np.arange(len(tbs))
                key = (pos // P) * G + tbs
                for kk in np.unique(key):
                    m = key == kk
                    s = ucols.setdefault(int(kk), set())
                    s.update(lcs[m].tolist())
            for kk in sorted(ucols):
                ch, tb = kk // G, kk % G
                colss = sorted(ucols[kk])
                i = 0
                while i < len(colss):
                    o = min(colss[i], P - W)
                    j = i
                    while j < len(colss) and colss[j] < o + W:
                        j += 1
                    jl.append((ch, tb, o))
                    i = j
            jobs[(q, h)] = jl
            so += len(jl)
            KSLOT = max(KSLOT, len(jl))
    NSLOT = int(so)

    idx_all = np.zeros((N_CORES, P, IDXC), np.int16)
    cm_all = np.full((N_CORES, P, NSLOT), SENT, np.float32)
    for c in range(N_CORES):
        for q in range(NQ):
            for h in range(2):
                srcs, tbs, lcs = streams[(c, q, h)]
                nqh = int(nidx_stat[q, h])
                stream = np.zeros(nqh, np.int64)
                stream[:len(srcs)] = srcs
                idx_all[c, :, idx_off[q, h]:idx_off[q, h] + idx_cols[q, h]] = \
                    _wrap16(stream.astype(np.int16))
                pos = np.arange(len(tbs))
                assigned = np.zeros(len(tbs), bool)
                for sl, (ch, tb, o) in enumerate(jobs[(q, h)]):
                    m = ((pos // P == ch) & (tbs == tb) & (lcs >= o)
                         & (lcs < o + W) & ~assigned)
                    if m.any():
                        assigned |= m
                        cm_all[c, pos[m] % P, slot_off[q, h] + sl] = lcs[m] - o

    x_t = np.zeros((IN_DIM, NP), np.float32)
    x_t[:, :n] = np.asarray(x, np.float32).T

    deg_ct = deg.reshape(N_CORES, T_C, P).transpose(0, 2, 1)  # [c, 128, T_C]
    deg_r = deg.reshape(N_CORES, 1, NSH)

    # iota for the windowed one-hot: value t at (p, t*KSLOT + k)
    iota_tk = np.tile(np.repeat(np.arange(W, dtype=np.float32), KSLOT)[None, :],
                      (P, 1))

    return dict(NP=NP, NSH=NSH, T_C=T_C, H0=H0, NQ=NQ,
                CQH=CQH, nidx_stat=nidx_stat, idx_cols=idx_cols,
                idx_off=idx_off, slot_off=slot_off, jobs=jobs,
                IDXC=IDXC, NSLOT=NSLOT, CMAXG=CMAXG, KSLOT=KSLOT,
                idx_all=idx_all, cm_all=cm_all, x_t=x_t, deg_ct=deg_ct,
                deg_r=deg_r, iota_tk=iota_tk)


def _build_program(meta, with_collective=True, act_lrelu=True):
    import concourse.bass as bass
    import concourse.bacc as bacc
    import concourse.mybir as mybir
    import concourse.tile as tile
    from concourse.masks import make_identity

    f32 = mybir.dt.float32
    bf16 = mybir.dt.bfloat16
    i16 = mybir.dt.int16
    AF = mybir.ActivationFunctionType
    Alu = mybir.AluOpType

    NSH, T_C, NP, H0 = meta["NSH"], meta["T_C"], meta["NP"], meta["H0"]
    NQ = meta["NQ"]
    CQH = meta["CQH"]
    nidx_stat = meta["nidx_stat"]
    idx_cols = meta["idx_cols"]
    idx_off = meta["idx_off"]
    slot_off = meta["slot_off"]
    jobs = meta["jobs"]
    IDXC, NSLOT, CMAXG, KSLOT = (meta["IDXC"], meta["NSLOT"], meta["CMAXG"],
                                 meta["KSLOT"])

    nc = bacc.Bacc("TRN2", target_bir_lowering=False, debug=False,
                   num_swdge_queues=4)
    table = nc.dram_tensor("cc_table", [NP, HID], bf16, addr_space="Shared")

    d_xq = nc.dram_tensor("xq", [P, 2 * NSH], bf16, kind="ExternalInput")
    d_wb = nc.dram_tensor("wb", [P, 6 * P + 3], bf16, kind="ExternalInput")
    d_bias = nc.dram_tensor("bias", [P, 6], f32, kind="ExternalInput")
    d_degct = nc.dram_tensor("deg_ct", [P, T_C], f32, kind="ExternalInput")
    d_degr = nc.dram_tensor("deg_r", [1, NSH], f32, kind="ExternalInput")
    d_idx = nc.dram_tensor("idx", [P, IDXC], i16, kind="ExternalInput")
    d_cmw = nc.dram_tensor("cmw", [P, NSLOT], bf16, kind="ExternalInput")
    d_iotk = nc.dram_tensor("iota_tk", [P, W * KSLOT], bf16,
                            kind="ExternalInput")
    d_out = nc.dram_tensor("outp", [OUT_DIM, NSH], f32, kind="ExternalOutput")

    WA, WB, WNOR, WAB, WATT, WUPD, WCLS, VH = (0, P, 2 * P, 3 * P, 4 * P,
                                               5 * P, 6 * P, 6 * P + 2)

    def emit_lrelu(out_ap, psum_ap, bias_ap, tmp_pool, nncols):
        if act_lrelu:
            nc.scalar.activation(out_ap, psum_ap, AF.Lrelu, bias=bias_ap,
                                 alpha=LRELU_SLOPE)
        else:
            nc.scalar.activation(out_ap, psum_ap, AF.Identity, bias=bias_ap)
            tl = tmp_pool.tile([P, 512], bf16, tag="lrtmp", name="lrtmp")
            nc.vector.tensor_scalar(out=tl[:, :nncols], in0=out_ap,
                                    scalar1=LRELU_SLOPE, scalar2=None,
                                    op0=Alu.mult)
            nc.vector.tensor_tensor(out=out_ap, in0=out_ap,
                                    in1=tl[:, :nncols], op=Alu.max)

    with tile.TileContext(nc) as tc:
        with (
            tc.tile_pool(name="const", bufs=1) as cpool,
            tc.tile_pool(name="sbuf", bufs=2) as pool,
            tc.tile_pool(name="sbe", bufs=2) as poole,
            tc.tile_pool(name="dram", bufs=1, space="DRAM") as dpool,
        ):
            # ---------- constants ----------
            wb = cpool.tile([P, 6 * P + 3], bf16)
            nc.sync.dma_start(wb[:], d_wb[:])
            bias = cpool.tile([P, 6], f32)
            nc.sync.dma_start(bias[:], d_bias[:])
            iotk = cpool.tile([P, W * KSLOT], bf16)
            nc.sync.dma_start(iotk[:], d_iotk[:])
            idx_sb = cpool.tile([P, IDXC], i16)
            nc.sync.dma_start(idx_sb[:], d_idx[:])
            cmw_sb = cpool.tile([P, NSLOT], bf16)
            nc.sync.dma_start(cmw_sb[:], d_cmw[:])

            identb = cpool.tile([P, P], bf16)
            make_identity(nc, identb[:])
            identb4 = cpool.tile([P, G * P], bf16)
            for j in range(G):
                nc.vector.tensor_scalar(out=identb4[:, j * P:(j + 1) * P],
                                        in0=identb[:], scalar1=1.0,
                                        scalar2=None, op0=Alu.mult)
            zerosb = cpool.tile([P, P], bf16)
            nc.vector.memset(zerosb[:], 0.0)
            ones1 = cpool.tile([1, P], bf16)
            nc.vector.memset(ones1[:], 1.0)

            dct = cpool.tile([P, T_C], f32)
            nc.sync.dma_start(dct[:], d_degct[:])
            nc.scalar.activation(dct[:], dct[:], AF.Sqrt)
            nc.vector.reciprocal(dct[:], dct[:])
            # flat bf16 dinv row [1, NSH]: PE-transpose dct, bounce via DRAM
            dctb = cpool.tile([P, P], bf16)
            nc.vector.memset(dctb[:], 1.0)
            nc.scalar.activation(dctb[:, :T_C], dct[:], AF.Identity)
            dinvr_b = cpool.tile([1, NSH], bf16)
            scr = dpool.tile([P, P], bf16)
            with tc.tile_pool(name="tps", bufs=1, space="PSUM") as tpp:
                ptc = tpp.tile([P, P], bf16)
                nc.tensor.transpose(ptc[:], dctb[:], identb[:])
                dctT = cpool.tile([P, P], bf16)
                nc.scalar.activation(dctT[:], ptc[:], AF.Identity)
                nc.sync.dma_start(scr[:], dctT[:])
                nc.sync.dma_start(
                    dinvr_b[:],
                    scr[:].rearrange("t p -> (t p)")[:NSH].unsqueeze(0))

            # persistent node-phase products (feature-major)
            m_fm = cpool.tile([P, NSH], bf16)     # m' table rows
            selfd = cpool.tile([P, NSH], bf16)    # dinv (x) m' (self term)
            dinvbb = cpool.tile([P, NSH], bf16)   # dinv[t] bcast per column

            shard = dpool.tile([NSH, HID], bf16)
            shard3 = shard[:].rearrange("(t p) f -> p t f", p=P)

            # ---------- node phase ----------
            with tc.tile_pool(name="npsum", bufs=4, space="PSUM") as npp, \
                 tc.tile_pool(name="nppal", bufs=2, space="PSUM") as nppal, \
                 tc.tile_pool(name="nptr", bufs=2, space="PSUM") as nptr:
                t0 = 0
                while t0 < T_C:
                    nt = min(SLAB, T_C - t0)
                    nn = nt * P
                    nb = t0 * P
                    pdb = npp.tile([P, 512], f32, tag="mm")
                    nc.tensor.matmul(pdb[:, :nn], ones1[:],
                                     dinvr_b[:, nb:nb + nn], start=True,
                                     stop=True)
                    nc.scalar.activation(dinvbb[:, nb:nb + nn], pdb[:, :nn],
                                         AF.Identity)
                    t0 += nt
                t0 = 0
                while t0 < T_C:
                    nt = min(SLAB, T_C - t0)
                    nn = nt * P
                    nn2 = (nt // 2) * P if nt > 1 else nn
                    nb = t0 * P
                    xsl = pool.tile([P, 2 * 512], bf16, tag="xsl")
                    nc.sync.dma_start(
                        xsl[:, :2 * nn].rearrange("p (a n) -> p a n", a=2),
                        d_xq[:].rearrange("p (a n) -> p a n",
                                          a=2)[:, :, nb:nb + nn])
                    ph = npp.tile([P, 512], f32, tag="mm")
                    nc.tensor.matmul(ph[:, :nn], wb[:, WA:WA + P],
                                     xsl[:, :nn], start=True, stop=False)
                    nc.tensor.matmul(ph[:, :nn], wb[:, WB:WB + P],
                                     xsl[:, nn:2 * nn],
                                     start=False, stop=True)
                    h = pool.tile([P, 512], bf16, tag="h")
                    emit_lrelu(h[:, :nn], ph[:, :nn], bias[:, 0:1], pool, nn)
                    pr = npp.tile([P, 512], f32, tag="mm")
                    nc.tensor.matmul(pr[:, :nn], wb[:, WNOR:WNOR + P],
                                     h[:, :nn], start=True, stop=True)
                    pd = npp.tile([P, 512], f32, tag="mm")
                    nc.tensor.matmul(pd[:, :nn], wb[:, WAB:WAB + P],
                                     h[:, :nn], start=True, stop=True)
                    patt = npp.tile([P, 512], f32, tag="mm")
                    nc.tensor.matmul(patt[:, :nn], wb[:, WATT:WATT + P],
                                     h[:, :nn], start=True, stop=True)
                    rr = pool.tile([P, 512], bf16, tag="rr")
                    nc.vector.tensor_scalar(out=rr[:, :nn], in0=pr[:, :nn],
                                            scalar1=bias[:, 1:2],
                                            scalar2=None, op0=Alu.add)
                    dh = pool.tile([P, 512], bf16, tag="dh")
                    nc.scalar.activation(dh[:, :nn], pd[:, :nn], AF.Identity,
                                         bias=bias[:, 2:3])
                    hatt = pool.tile([P, 512], bf16, tag="hatt")
                    nc.scalar.activation(hatt[:, :nn], patt[:, :nn], AF.Tanh,
                                         bias=bias[:, 3:4])
                    # alpha row: sigmoid(q) = 0.5*tanh(q/2)+0.5 (v pre-halved)
                    pal = nppal.tile([1, 512], f32, tag="pal")
                    nc.tensor.matmul(pal[:, :nn], wb[:, VH:VH + 1],
                                     hatt[:, :nn], start=True, stop=True)
                    sig = pool.tile([1, 512], bf16, tag="sig")
                    nc.scalar.activation(sig[:, :nn], pal[:, :nn], AF.Tanh)
                    psg = npp.tile([P, 512], f32, tag="mm")
                    nc.tensor.matmul(psg[:, :nn], ones1[:], sig[:, :nn],
                                     start=True, stop=True)
                    sgb = pool.tile([P, 512], bf16, tag="sgb")
                    nc.vector.tensor_scalar(out=sgb[:, :nn], in0=psg[:, :nn],
                                            scalar1=1.0, scalar2=None,
                                            op0=Alu.mult)
                    s2 = pool.tile([P, 512], bf16, tag="s2")
                    nc.vector.tensor_mul(s2[:, :nn], dh[:, :nn], sgb[:, :nn])
                    ms = pool.tile([P, 512], bf16, tag="ms")
                    nc.vector.tensor_add(ms[:, :nn], rr[:, :nn], s2[:, :nn])
                    mfs = pool.tile([P, 512], bf16, tag="mfs")
                    nc.vector.tensor_mul(mfs[:, :nn], ms[:, :nn],
                                         dinvbb[:, nb:nb + nn])
                    nc.vector.tensor_scalar(out=m_fm[:, nb:nb + nn],
                                            in0=mfs[:, :nn], scalar1=1.0,
                                            scalar2=None, op0=Alu.mult)
                    nc.vector.tensor_mul(selfd[:, nb:nb + nn], mfs[:, :nn],
                                         dinvbb[:, nb:nb + nn])
                    # node-major rows for the gather table
                    ptm = nptr.tile([P, 512], bf16, tag="tr")
                    for j in range(nt):
                        nc.tensor.transpose(
                            ptm[:, j * P:(j + 1) * P],
                            mfs[:, j * P:(j + 1) * P], identb[:])
                    mrow = pool.tile([P, 512], bf16, tag="mrow")
                    nc.scalar.activation(mrow[:, :nn2], ptm[:, :nn2],
                                         AF.Identity)
                    if nn > nn2:
                        nc.vector.tensor_scalar(out=mrow[:, nn2:nn],
                                                in0=ptm[:, nn2:nn],
                                                scalar1=1.0, scalar2=None,
                                                op0=Alu.mult)
                    nc.sync.dma_start(
                        shard3[:, t0:t0 + nt, :],
                        mrow[:, :nn].rearrange("p (t f) -> p t f", f=P))
                    t0 += nt
            # ---------- replicate table ----------
            if with_collective:
                nc.gpsimd.collective_compute(
                    "AllGather",
                    mybir.AluOpType.bypass,
                    replica_groups=[list(range(N_CORES))],
                    ins=[shard.opt()],
                    outs=[table[:]],
                )
            else:
                nc.sync.dma_start(table[:NSH, :], shard[:])

            # ---------- edge phase ----------
            with tc.tile_pool(name="epsum", bufs=2, space="PSUM") as epp, \
                 tc.tile_pool(name="eps2", bufs=2, space="PSUM") as epp2:
                for q in range(NQ):
                    ntile = min(G, T_C - q * G)
                    nn = ntile * P
                    gb = []
                    sohs = []
                    for hh in range(2):
                        C = int(CQH[q, hh])
                        nix = int(nidx_stat[q, hh])
                        nj = len(jobs[(q, hh)])
                        g = poole.tile([P, CMAXG * HID], bf16, tag=f"g{hh}")
                        soh = poole.tile([P, W * KSLOT], bf16, tag=f"soh{hh}")
                        gb.append(g)
                        sohs.append(soh)
                        if nix == 0:
                            continue
                        # SWDGE ring caps one gather at 1024 descriptors
                        c0 = 0
                        while c0 < C:
                            cs_ = min(8, C - c0)
                            ni = cs_ * P
                            nc.gpsimd.dma_gather(
                                out_ap=g[:, c0 * HID:(c0 + cs_) * HID]
                                .rearrange("p (c f) -> p c f", f=HID),
                                in_ap=table[hh * H0:hh * H0 + H0, :],
                                idxs_ap=idx_sb[:, idx_off[q, hh] + c0 * 8:
                                               idx_off[q, hh]
                                               + (c0 + cs_) * 8],
                                num_idxs=ni,
                                num_idxs_reg=ni,
                                elem_size=HID,
                                queue_num=(2 * q + hh + c0 // 8) % 4,
                            )
                            c0 += cs_
                        if nj:
                            nc.vector.tensor_tensor(
                                out=soh[:, :W * nj].rearrange(
                                    "p (t k) -> p t k", k=nj),
                                in0=iotk[:].rearrange(
                                    "p (t k) -> p t k", k=KSLOT)[:, :, :nj],
                                in1=cmw_sb[:, slot_off[q, hh]:
                                           slot_off[q, hh] + nj]
                                .unsqueeze(1).to_broadcast([P, W, nj]),
                                op=Alu.is_equal)
                    pagg = epp.tile([P, G * P], f32, tag="pagg")
                    nc.tensor.matmul(pagg[:, :nn], zerosb[:],
                                     identb4[:, :nn], start=True, stop=False)
                    for hh in range(2):
                        nj = len(jobs[(q, hh)])
                        if nj == 0:
                            continue
                        soh3 = sohs[hh][:, :W * nj].rearrange(
                            "p (t k) -> p t k", k=nj)
                        for sl, (ch, tb, o) in enumerate(jobs[(q, hh)]):
                            ob = tb * P + o
                            nc.tensor.matmul(
                                pagg[:, ob:ob + W],
                                gb[hh][:, ch * HID:(ch + 1) * HID],
                                soh3[:, :, sl], start=False, stop=False)
                    nc.tensor.matmul(pagg[:, :nn], zerosb[:],
                                     identb4[:, :nn], start=False, stop=True)
                    qb = q * G * P
                    paggb = pool.tile([P, G * P], bf16, tag="paggb")
                    nc.scalar.activation(paggb[:, :nn], pagg[:, :nn],
                                         AF.Identity)
                    m2a = pool.tile([P, G * P], bf16, tag="m2a")
                    nc.vector.tensor_mul(m2a[:, :nn], paggb[:, :nn],
                                         dinvbb[:, qb:qb + nn])
                    m2b = pool.tile([P, G * P], bf16, tag="m2b")
                    nc.vector.tensor_add(m2b[:, :nn], m2a[:, :nn],
                                         selfd[:, qb:qb + nn])
                    pu = epp2.tile([P, G * P], f32, tag="pu")
                    nc.tensor.matmul(pu[:, :nn], wb[:, WUPD:WUPD + P],
                                     m2b[:, :nn], start=True, stop=True)
                    lu = pool.tile([P, G * P], bf16, tag="lu")
                    emit_lrelu(lu[:, :nn], pu[:, :nn], bias[:, 4:5], pool, nn)
                    po = epp2.tile([OUT_DIM, G * P], f32, tag="po")
                    nc.tensor.matmul(po[:, :nn], wb[:, WCLS:WCLS + OUT_DIM],
                                     lu[:, :nn], start=True, stop=True)
                    ot = pool.tile([OUT_DIM, G * P], f32, tag="ot")
                    nc.scalar.activation(ot[:, :nn], po[:, :nn], AF.Identity,
                                         bias=bias[0:OUT_DIM, 5:6])
                    nc.sync.dma_start(d_out[:, qb:qb + nn], ot[:, :nn])

    nc.compile()
    return nc


def _run_spmd_presharded(nc, in_maps, n_cores=N_CORES):
    """Run a compiled Bass program on n_cores via PJRT with host-side
    pre-sharded inputs (avoids XLA reshard programs on big arrays)."""
    import jax
    import concourse.mybir as mybir
    from concourse import bass2jax
    from jax.sharding import Mesh, PartitionSpec, NamedSharding
    from jax.experimental.shard_map import shard_map

    bass2jax.install_neuronx_cc_hook()
    partition_name = nc.partition_id_tensor.name if nc.partition_id_tensor else None
    in_names, out_names, out_avals, zero_outs = [], [], [], []
    for alloc in nc.m.functions[0].allocations:
        if not isinstance(alloc, mybir.MemoryLocationSet):
            continue
        name = alloc.memorylocations[0].name
        if alloc.kind == "ExternalInput":
            if name != partition_name:
                in_names.append(name)
        elif alloc.kind == "ExternalOutput":
            out_names.append(name)
            shape = tuple(alloc.tensor_shape)
            dtype = mybir.dt.np(alloc.dtype)
            out_avals.append(jax.core.ShapedArray(shape, dtype))
            zero_outs.append(np.zeros(shape, dtype))
    n_params = len(in_names)
    in_names_all = list(in_names) + out_names
    if partition_name is not None:
        in_names_all.append(partition_name)

    def _body(*args):
        operands = list(args)
        if partition_name is not None:
            operands.append(bass2jax.partition_id_tensor())
        outs = bass2jax._bass_exec_p.bind(
            *operands,
            out_avals=tuple(out_avals),
            in_names=tuple(in_names_all),
            out_names=tuple(out_names),
            lowering_input_output_aliases=(),
            sim_require_finite=True,
            sim_require_nnan=True,
            nc=nc,
        )
        return tuple(outs)

    devices = jax.devices()[:n_cores]
    mesh = Mesh(np.asarray(devices), ("core",))
    spec = PartitionSpec("core")
    n_outs = len(out_avals)
    sharded = jax.jit(
        shard_map(_body, mesh=mesh, in_specs=(spec,) * (n_params + n_outs),
                  out_specs=(spec,) * n_outs, check_rep=False),
        keep_unused=True,
    )
    sh = NamedSharding(mesh, spec)

    def put(per_core_arrays):
        a0 = np.asarray(per_core_arrays[0])
        gshape = (n_cores * a0.shape[0],) + a0.shape[1:]
        shards = [jax.device_put(np.ascontiguousarray(per_core_arrays[c]),
                                 devices[c]) for c in range(n_cores)]
        return jax.make_array_from_single_device_arrays(gshape, sh, shards)

    args = [put([m[name] for m in in_maps]) for name in in_names]
    args += [put([z] * n_cores) for z in zero_outs]
    out_arrs = sharded(*args)
    jax.block_until_ready(out_arrs)
    return [
        {name: np.asarray(out_arrs[i]).reshape(n_cores, *out_avals[i].shape)[c]
         for i, name in enumerate(out_names)}
        for c in range(n_cores)
    ]


def kernel(x, edge_index, W_in, b_in, W_nor, b_nor, W_abnor, b_abnor,
           W_att, b_att, v_att, W_upd, b_upd, W_cls, b_cls):
    import ml_dtypes
    bf = ml_dtypes.bfloat16

    x = np.asarray(x, np.float32)
    n = x.shape[0]
    meta = _host_plan(x, edge_index)
    NSH = meta["NSH"]
    nc = _build_program(meta, with_collective=True)

    Wnor_p = np.zeros((P, P), np.float32)
    Wnor_p[:HALF] = np.asarray(W_nor, np.float32)
    Wab_p = np.zeros((P, P), np.float32)
    Wab_p[HALF:] = np.asarray(W_abnor, np.float32)
    b_nor_ = np.asarray(b_nor, np.float32)
    b_ab_ = np.asarray(b_abnor, np.float32)
    W_st = Wnor_p + Wab_p
    b_st = b_nor_ + b_ab_
    wbk = np.zeros((P, 6 * P + 3), np.float32)
    wbk[:, 0:P] = np.asarray(W_in, np.float32)[:P, :]
    wbk[:, P:2 * P] = np.asarray(W_in, np.float32)[P:, :]
    wbk[:, 2 * P:3 * P] = 0.5 * W_st                       # r = 0.5(xn+xa)
    wbk[:, 3 * P:4 * P] = 0.5 * (Wnor_p - Wab_p)           # ddh = 0.5(xn-xa)
    wbk[:, 4 * P:5 * P] = W_st @ np.asarray(W_att, np.float32)
    wbk[:, 5 * P:6 * P] = np.asarray(W_upd, np.float32)
    wbk[:, 6 * P:6 * P + 2] = np.asarray(W_cls, np.float32)
    wbk[:, 6 * P + 2] = np.asarray(v_att, np.float32).reshape(-1) * 0.5

    bias = np.zeros((P, 6), np.float32)
    bias[:, 0] = np.asarray(b_in, np.float32)
    bias[:, 1] = 0.5 * b_st
    bias[:, 2] = 0.5 * (b_nor_ - b_ab_)
    bias[:, 3] = (b_st @ np.asarray(W_att, np.float32)
                  + np.asarray(b_att, np.float32))
    bias[:, 4] = np.asarray(b_upd, np.float32)
    bias[:OUT_DIM, 5] = np.asarray(b_cls, np.float32)

    shared = {
        "wb": wbk.astype(bf),
        "bias": bias,
        "iota_tk": meta["iota_tk"].astype(bf),
    }
    x_t = meta["x_t"]
    in_maps = []
    for c in range(N_CORES):
        xc = np.concatenate([x_t[:P, c * NSH:(c + 1) * NSH],
                             x_t[P:, c * NSH:(c + 1) * NSH]], axis=1)
        in_maps.append({
            **shared,
            "xq": np.ascontiguousarray(xc).astype(bf),
            "deg_ct": meta["deg_ct"][c],
            "deg_r": meta["deg_r"][c],
            "idx": meta["idx_all"][c],
            "cmw": meta["cm_all"][c].astype(bf),
        })

    results = _run_spmd_presharded(nc, in_maps)
    out_full = np.empty((meta["NP"], OUT_DIM), np.float32)
    for c in range(N_CORES):
        out_full[c * NSH:(c + 1) * NSH] = results[c]["outp"].T
    return np.ascontiguousarray(out_full[:n])


# revision 28
# speedup vs baseline: 1.5070x; 1.1058x over previous
"""CGNN message-passing kernel for Trainium2, 8 NeuronCores.

Strategy (v3):
  - Algebraic reduction: the attention gate depends only on the source node,
    so the edge computation collapses to aggr = dinv (x) (A @ m') with
    m'[j] = dinv_j*(alpha_j*xn_j + (1-alpha_j)*xa_j) a per-node table.
  - Contiguous node sharding: core c owns nodes [c*NSH, (c+1)*NSH) for both
    phases; self-loop terms are added from SBUF (no gathering of self rows).
  - Node phase is fully feature-major and slab-batched: the alpha logit is a
    [1, n] row matmul, alpha*dinv and dinv broadcasts are outer-product
    matmuls, the message mix is three slab-wide DVE ops; only the final
    node-major table rows need a PE transpose per tile. Sigmoid is realized
    as 0.5*tanh(q/2)+0.5 (v_att pre-halved) so Lrelu/Tanh/Identity share one
    activation table.
  - Edge phase: edges sorted by (4-tile quad, table half, target); one SWDGE
    dma_gather per (quad, half) with exact max-count index streams. The
    scatter-add is PSUM-accumulated matmuls over a narrow one-hot: chunks of
    128 target-sorted edges span only ~8 targets, so each matmul writes a
    32-column window of the packed [128, 512] quad PSUM. Window offsets are
    compile-time (union over cores, computed per run); per-core variability
    lives in the cm data (sentinels mask pad slots). The one-hot is built in
    [p, t, k] layout against a materialized iota so all DVE operands are
    packed 2-byte -> 2x DVE rate.
"""
import numpy as np

N_CORES = 8
P = 128
IN_DIM = 256
HID = 128
HALF = 64
OUT_DIM = 2
LRELU_SLOPE = 0.01
SLAB = 4                  # node-phase tiles per slab (<=512 cols)
G = 4                     # edge-phase tiles per quad
W = 32                    # one-hot window width
SENT = 200.0              # one-hot sentinel (never matches a window col)


def _wrap16(flat):
    """[n] int16 -> [128, ceil(n/16)] in the SWDGE 16-wrap + 8x replicated
    layout (flat position i lands at row i%16, col i//16)."""
    n = len(flat)
    cols = -(-n // 16)
    pad = np.zeros(cols * 16, np.int16)
    pad[:n] = flat
    blk = pad.reshape(cols, 16).T          # [16, cols]
    return np.tile(blk, (8, 1))            # [128, cols]


def _host_plan(x, edge_index):
    n = x.shape[0]
    NP = ((n + 1023) // 1024) * 1024       # 50176
    NSH = NP // N_CORES                    # 6272
    T_C = NSH // P                         # 49
    SPL_T = max(SLAB, (T_C // 2 // SLAB) * SLAB)   # 24 tiles -> table A
    SPA = SPL_T * P                        # 3072 rows/core in table A
    SPB = NSH - SPA                        # 3200 rows/core in table B
    NQ = -(-T_C // G)                      # 13 quads (last has 1 tile)

    ei = np.asarray(edge_index)
    row = ei[0].astype(np.int64)
    col = ei[1].astype(np.int64)

    deg = np.bincount(col, minlength=NP).astype(np.float32)
    deg[:n] += 1.0                         # self loops
    deg[n:] = 1.0                          # pad nodes stay finite

    core_e = col // NSH
    r_e = row % NSH
    half_e = (r_e >= SPA).astype(np.int64)
    src_c = row // NSH
    src_remap = np.where(half_e == 0, src_c * SPA + r_e,
                         src_c * SPB + (r_e - SPA))
    lt_e = (col % NSH) // P                # local tile 0..48
    q_e = lt_e // G

    # per (core, quad, half) edge counts -> static stream sizes
    cnt = np.zeros((N_CORES, NQ, 2), np.int64)
    np.add.at(cnt, (core_e, q_e, half_e), 1)
    nmax = cnt.max(axis=0)                 # [NQ, 2]
    nidx_stat = ((nmax + 127) // 128) * 128  # num_idxs per gather
    CQH = -(-nidx_stat // P)               # gather chunks
    idx_cols = nidx_stat // 16
    idx_off = np.zeros((NQ, 2), np.int64)
    io = 0
    for q in range(NQ):
        for h in range(2):
            idx_off[q, h] = io
            io += idx_cols[q, h]
    IDXC = int(io)
    CMAXG = int(CQH.max())

    order = np.lexsort((col, half_e, q_e, core_e))
    rms = src_remap[order]
    cs = col[order]
    bounds = np.concatenate([[0], np.cumsum(cnt.reshape(-1))])

    # per-core streams: sources, tile-in-quad, local col
    streams = {}
    for c in range(N_CORES):
        for q in range(NQ):
            for h in range(2):
                bi = (c * NQ + q) * 2 + h
                b0, b1 = bounds[bi], bounds[bi + 1]
                streams[(c, q, h)] = (rms[b0:b1],
                                      (cs[b0:b1] % NSH) // P - q * G,
                                      (cs[b0:b1] % P))

    # jobs: per (q, h): [(chunk, tile-in-quad, window offset)], windows from
    # the union of target cols over cores (exact, computed per run)
    jobs = {}
    slot_off = np.zeros((NQ, 2), np.int64)
    so = 0
    KSLOT = 0
    for q in range(NQ):
        for h in range(2):
            slot_off[q, h] = so
            jl = []
            ucols = {}
            for c in range(N_CORES):
                _, tbs, lcs = streams[(c, q, h)]
                pos = np.arange(len(tbs))
                key = (pos // P) * G + tbs
                for kk in np.unique(key):
                    m = key == kk
                    s = ucols.setdefault(int(kk), set())
                    s.update(lcs[m].tolist())
            for kk in sorted(ucols):
                ch, tb = kk // G, kk % G
                colss = sorted(ucols[kk])
                i = 0
                while i < len(colss):
                    o = min(colss[i], P - W)
                    j = i
                    while j < len(colss) and colss[j] < o + W:
                        j += 1
                    jl.append((ch, tb, o))
                    i = j
            jobs[(q, h)] = jl
            so += len(jl)
            KSLOT = max(KSLOT, len(jl))
    NSLOT = int(so)

    idx_all = np.zeros((N_CORES, P, IDXC), np.int16)
    cm_all = np.full((N_CORES, P, NSLOT), SENT, np.float32)
    for c in range(N_CORES):
        for q in range(NQ):
            for h in range(2):
                srcs, tbs, lcs = streams[(c, q, h)]
                nqh = int(nidx_stat[q, h])
                stream = np.zeros(nqh, np.int64)
                stream[:len(srcs)] = srcs
                idx_all[c, :, idx_off[q, h]:idx_off[q, h] + idx_cols[q, h]] = \
                    _wrap16(stream.astype(np.int16))
                pos = np.arange(len(tbs))
                assigned = np.zeros(len(tbs), bool)
                for sl, (ch, tb, o) in enumerate(jobs[(q, h)]):
                    m = ((pos // P == ch) & (tbs == tb) & (lcs >= o)
                         & (lcs < o + W) & ~assigned)
                    if m.any():
                        assigned |= m
                        cm_all[c, pos[m] % P, slot_off[q, h] + sl] = lcs[m] - o

    x_t = np.zeros((IN_DIM, NP), np.float32)
    x_t[:, :n] = np.asarray(x, np.float32).T

    deg_ct = deg.reshape(N_CORES, T_C, P).transpose(0, 2, 1)  # [c, 128, T_C]
    deg_r = deg.reshape(N_CORES, 1, NSH)

    # iota for the windowed one-hot: value t at (p, t*KSLOT + k)
    iota_tk = np.tile(np.repeat(np.arange(W, dtype=np.float32), KSLOT)[None, :],
                      (P, 1))

    return dict(NP=NP, NSH=NSH, T_C=T_C, SPL_T=SPL_T, SPA=SPA, SPB=SPB,
                NQ=NQ,
                CQH=CQH, nidx_stat=nidx_stat, idx_cols=idx_cols,
                idx_off=idx_off, slot_off=slot_off, jobs=jobs,
                IDXC=IDXC, NSLOT=NSLOT, CMAXG=CMAXG, KSLOT=KSLOT,
                idx_all=idx_all, cm_all=cm_all, x_t=x_t, deg_ct=deg_ct,
                deg_r=deg_r, iota_tk=iota_tk)


def _build_program(meta, with_collective=True, act_lrelu=True):
    import concourse.bass as bass
    import concourse.bacc as bacc
    import concourse.mybir as mybir
    import concourse.tile as tile
    from concourse.masks import make_identity

    f32 = mybir.dt.float32
    bf16 = mybir.dt.bfloat16
    i16 = mybir.dt.int16
    AF = mybir.ActivationFunctionType
    Alu = mybir.AluOpType

    NSH, T_C, NP = meta["NSH"], meta["T_C"], meta["NP"]
    SPL_T, SPA, SPB = meta["SPL_T"], meta["SPA"], meta["SPB"]
    NQ = meta["NQ"]
    CQH = meta["CQH"]
    nidx_stat = meta["nidx_stat"]
    idx_cols = meta["idx_cols"]
    idx_off = meta["idx_off"]
    slot_off = meta["slot_off"]
    jobs = meta["jobs"]
    IDXC, NSLOT, CMAXG, KSLOT = (meta["IDXC"], meta["NSLOT"], meta["CMAXG"],
                                 meta["KSLOT"])

    nc = bacc.Bacc("TRN2", target_bir_lowering=False, debug=False,
                   num_swdge_queues=4)
    table_a = nc.dram_tensor("cc_table_a", [N_CORES * SPA, HID], bf16,
                             addr_space="Shared")
    table_b = nc.dram_tensor("cc_table_b", [N_CORES * SPB, HID], bf16,
                             addr_space="Shared")

    d_xq = nc.dram_tensor("xq", [P, 2 * NSH], bf16, kind="ExternalInput")
    d_wb = nc.dram_tensor("wb", [P, 6 * P + 3], bf16, kind="ExternalInput")
    d_bias = nc.dram_tensor("bias", [P, 6], f32, kind="ExternalInput")
    d_degct = nc.dram_tensor("deg_ct", [P, T_C], f32, kind="ExternalInput")
    d_degr = nc.dram_tensor("deg_r", [1, NSH], f32, kind="ExternalInput")
    d_idx = nc.dram_tensor("idx", [P, IDXC], i16, kind="ExternalInput")
    d_cmw = nc.dram_tensor("cmw", [P, NSLOT], bf16, kind="ExternalInput")
    d_iotk = nc.dram_tensor("iota_tk", [P, W * KSLOT], bf16,
                            kind="ExternalInput")
    d_bddh = nc.dram_tensor("b_ddh_r", [1, P], bf16, kind="ExternalInput")
    d_out = nc.dram_tensor("outp", [OUT_DIM, NSH], f32, kind="ExternalOutput")

    WA, WB, WNOR, WAB, WATT, WUPD, WCLS, VH = (0, P, 2 * P, 3 * P, 4 * P,
                                               5 * P, 6 * P, 6 * P + 2)

    def emit_lrelu(out_ap, psum_ap, bias_ap, tmp_pool, nncols):
        if act_lrelu:
            nc.scalar.activation(out_ap, psum_ap, AF.Lrelu, bias=bias_ap,
                                 alpha=LRELU_SLOPE)
        else:
            nc.scalar.activation(out_ap, psum_ap, AF.Identity, bias=bias_ap)
            tl = tmp_pool.tile([P, 512], bf16, tag="lrtmp", name="lrtmp")
            nc.vector.tensor_scalar(out=tl[:, :nncols], in0=out_ap,
                                    scalar1=LRELU_SLOPE, scalar2=None,
                                    op0=Alu.mult)
            nc.vector.tensor_tensor(out=out_ap, in0=out_ap,
                                    in1=tl[:, :nncols], op=Alu.max)

    with tile.TileContext(nc) as tc:
        with (
            tc.tile_pool(name="const", bufs=1) as cpool,
            tc.tile_pool(name="sbuf", bufs=2) as pool,
            tc.tile_pool(name="sbe", bufs=2) as poole,
            tc.tile_pool(name="dram", bufs=1, space="DRAM") as dpool,
        ):
            # ---------- constants ----------
            wb = cpool.tile([P, 6 * P + 3], bf16)
            nc.sync.dma_start(wb[:], d_wb[:])
            bias = cpool.tile([P, 6], f32)
            nc.sync.dma_start(bias[:], d_bias[:])
            iotk = cpool.tile([P, W * KSLOT], bf16)
            nc.sync.dma_start(iotk[:], d_iotk[:])
            idx_sb = cpool.tile([P, IDXC], i16)
            nc.sync.dma_start(idx_sb[:], d_idx[:])
            cmw_sb = cpool.tile([P, NSLOT], bf16)
            nc.sync.dma_start(cmw_sb[:], d_cmw[:])

            identb = cpool.tile([P, P], bf16)
            make_identity(nc, identb[:])
            identb4 = cpool.tile([P, G * P], bf16)
            for j in range(G):
                nc.vector.tensor_scalar(out=identb4[:, j * P:(j + 1) * P],
                                        in0=identb[:], scalar1=1.0,
                                        scalar2=None, op0=Alu.mult)
            zerosb = cpool.tile([P, P], bf16)
            nc.vector.memset(zerosb[:], 0.0)
            ones1 = cpool.tile([1, P], bf16)
            nc.vector.memset(ones1[:], 1.0)
            onesr = cpool.tile([1, 512], bf16)
            nc.vector.memset(onesr[:], 1.0)
            bddh = cpool.tile([1, P], bf16)
            nc.sync.dma_start(bddh[:], d_bddh[:])

            dct = cpool.tile([P, T_C], f32)
            nc.sync.dma_start(dct[:], d_degct[:])
            nc.scalar.activation(dct[:], dct[:], AF.Sqrt)
            nc.vector.reciprocal(dct[:], dct[:])
            # flat bf16 dinv row [1, NSH]: PE-transpose dct, bounce via DRAM
            dctb = cpool.tile([P, P], bf16)
            nc.vector.memset(dctb[:], 1.0)
            nc.scalar.activation(dctb[:, :T_C], dct[:], AF.Identity)
            dinvr_b = cpool.tile([1, NSH], bf16)
            scr = dpool.tile([P, P], bf16)
            with tc.tile_pool(name="tps", bufs=1, space="PSUM") as tpp:
                ptc = tpp.tile([P, P], bf16)
                nc.tensor.transpose(ptc[:], dctb[:], identb[:])
                dctT = cpool.tile([P, P], bf16)
                nc.scalar.activation(dctT[:], ptc[:], AF.Identity)
                nc.sync.dma_start(scr[:], dctT[:])
                nc.sync.dma_start(
                    dinvr_b[:],
                    scr[:].rearrange("t p -> (t p)")[:NSH].unsqueeze(0))

            # persistent node-phase products (feature-major)
            selfd = cpool.tile([P, NSH], bf16)    # dinv (x) m' (self term)
            dinvbb = cpool.tile([P, NSH], bf16)   # dinv[t] bcast per column

            shard_a = dpool.tile([SPA, HID], bf16)
            shard_b = dpool.tile([SPB, HID], bf16)
            shard3a = shard_a[:].rearrange("(t p) f -> p t f", p=P)
            shard3b = shard_b[:].rearrange("(t p) f -> p t f", p=P)
            # spill for phase-A partial aggregates
            spill = cpool.tile([P, G * P * NQ], bf16)

            # ---------- node phase ----------
            with tc.tile_pool(name="npsum", bufs=4, space="PSUM") as npp, \
                 tc.tile_pool(name="nppal", bufs=2, space="PSUM") as nppal, \
                 tc.tile_pool(name="nptr", bufs=2, space="PSUM") as nptr:
                t0 = 0
                while t0 < T_C:
                    nt = min(SLAB, T_C - t0)
                    nn = nt * P
                    nb = t0 * P
                    pdb = npp.tile([P, 512], f32, tag="mm")
                    nc.tensor.matmul(pdb[:, :nn], ones1[:],
                                     dinvr_b[:, nb:nb + nn], start=True,
                                     stop=True)
                    nc.scalar.activation(dinvbb[:, nb:nb + nn], pdb[:, :nn],
                                         AF.Identity)
                    t0 += nt
                slabs = []
                t0 = 0
                while t0 < T_C:
                    nt = min(SLAB, T_C - t0)
                    slabs.append((t0, nt))
                    t0 += nt
                SPL_SLAB = SPL_T // SLAB
                state = {}

                def stage1(i):
                    t0, nt = slabs[i]
                    nn = nt * P
                    nb = t0 * P
                    xsl = pool.tile([P, 2 * 512], bf16, tag="xsl",
                                    name="xsl")
                    nc.sync.dma_start(
                        xsl[:, :2 * nn].rearrange("p (a n) -> p a n", a=2),
                        d_xq[:].rearrange("p (a n) -> p a n",
                                          a=2)[:, :, nb:nb + nn])
                    ph = npp.tile([P, 512], f32, tag="mm", name="ph")
                    nc.tensor.matmul(ph[:, :nn], wb[:, WA:WA + P],
                                     xsl[:, :nn], start=True, stop=False)
                    nc.tensor.matmul(ph[:, :nn], wb[:, WB:WB + P],
                                     xsl[:, nn:2 * nn],
                                     start=False, stop=True)
                    h = pool.tile([P, 512], bf16, tag="h", name="h")
                    emit_lrelu(h[:, :nn], ph[:, :nn], bias[:, 0:1], pool, nn)
                    pr = npp.tile([P, 512], f32, tag="mm", name="pr")
                    nc.tensor.matmul(pr[:, :nn], wb[:, WNOR:WNOR + P],
                                     h[:, :nn], start=True, stop=True)
                    pd = npp.tile([P, 512], f32, tag="mm", name="pd")
                    nc.tensor.matmul(pd[:, :nn], wb[:, WAB:WAB + P],
                                     h[:, :nn], start=True, stop=False)
                    nc.tensor.matmul(pd[:, :nn], bddh[:], onesr[:, :nn],
                                     start=False, stop=True)
                    patt = npp.tile([P, 512], f32, tag="mm", name="patt")
                    nc.tensor.matmul(patt[:, :nn], wb[:, WATT:WATT + P],
                                     h[:, :nn], start=True, stop=True)
                    rr = pool.tile([P, 512], bf16, tag="rr", name="rr")
                    nc.vector.tensor_scalar(out=rr[:, :nn], in0=pr[:, :nn],
                                            scalar1=bias[:, 1:2],
                                            scalar2=None, op0=Alu.add)
                    hatt = pool.tile([P, 512], bf16, tag="hatt", name="hatt")
                    nc.scalar.activation(hatt[:, :nn], patt[:, :nn], AF.Tanh,
                                         bias=bias[:, 3:4])
                    pal = nppal.tile([1, 512], f32, tag="pal", name="pal")
                    nc.tensor.matmul(pal[:, :nn], wb[:, VH:VH + 1],
                                     hatt[:, :nn], start=True, stop=True)
                    sig = pool.tile([1, 512], bf16, tag="sig", name="sig")
                    nc.scalar.activation(sig[:, :nn], pal[:, :nn], AF.Tanh)
                    state[i] = (rr, pd, sig)

                def stage2(i):
                    t0, nt = slabs[i]
                    nn = nt * P
                    nn2 = (nt // 2) * P if nt > 1 else nn
                    nb = t0 * P
                    rr, pd, sig = state.pop(i)
                    psg = npp.tile([P, 512], f32, tag="mm", name="psg")
                    nc.tensor.matmul(psg[:, :nn], ones1[:], sig[:, :nn],
                                     start=True, stop=True)
                    sgb = pool.tile([P, 512], f32, tag="sgb", name="sgb")
                    nc.scalar.activation(sgb[:, :nn], psg[:, :nn],
                                         AF.Identity)
                    s2 = pool.tile([P, 512], bf16, tag="s2", name="s2")
                    nc.vector.tensor_mul(s2[:, :nn], pd[:, :nn],
                                         sgb[:, :nn])
                    ms = pool.tile([P, 512], bf16, tag="ms", name="ms")
                    nc.vector.tensor_add(ms[:, :nn], rr[:, :nn], s2[:, :nn])
                    mfs = pool.tile([P, 512], bf16, tag="mfs", name="mfs")
                    nc.vector.tensor_mul(mfs[:, :nn], ms[:, :nn],
                                         dinvbb[:, nb:nb + nn])
                    nc.vector.tensor_mul(selfd[:, nb:nb + nn], mfs[:, :nn],
                                         dinvbb[:, nb:nb + nn])
                    ptm = nptr.tile([P, 512], bf16, tag="tr", name="tr")
                    for j in range(nt):
                        nc.tensor.transpose(
                            ptm[:, j * P:(j + 1) * P],
                            mfs[:, j * P:(j + 1) * P], identb[:])
                    mrow = pool.tile([P, 512], bf16, tag="mrow", name="mrow")
                    nc.scalar.activation(mrow[:, :nn2], ptm[:, :nn2],
                                         AF.Identity)
                    if nn > nn2:
                        nc.vector.tensor_scalar(out=mrow[:, nn2:nn],
                                                in0=ptm[:, nn2:nn],
                                                scalar1=1.0, scalar2=None,
                                                op0=Alu.mult)
                    if t0 < SPL_T:
                        nc.sync.dma_start(
                            shard3a[:, t0:t0 + nt, :],
                            mrow[:, :nn].rearrange("p (t f) -> p t f", f=P))
                    else:
                        nc.sync.dma_start(
                            shard3b[:, t0 - SPL_T:t0 - SPL_T + nt, :],
                            mrow[:, :nn].rearrange("p (t f) -> p t f", f=P))

                for i in range(len(slabs)):
                    stage1(i)
                    if i > 0:
                        stage2(i - 1)
                    if i == SPL_SLAB:
                        if with_collective:
                            nc.gpsimd.collective_compute(
                                "AllGather", mybir.AluOpType.bypass,
                                replica_groups=[list(range(N_CORES))],
                                ins=[shard_a.opt()], outs=[table_a[:]])
                        else:
                            nc.sync.dma_start(table_a[:SPA, :], shard_a[:])
                stage2(len(slabs) - 1)

            # ---------- replicate table (half B) ----------
            if with_collective:
                nc.gpsimd.collective_compute(
                    "AllGather",
                    mybir.AluOpType.bypass,
                    replica_groups=[list(range(N_CORES))],
                    ins=[shard_b.opt()],
                    outs=[table_b[:]],
                )
            else:
                nc.sync.dma_start(table_b[:SPB, :], shard_b[:])

            # ---------- edge phase ----------
            with tc.tile_pool(name="epsum", bufs=3, space="PSUM") as epp, \
                 tc.tile_pool(name="eps2", bufs=2, space="PSUM") as epp2:

                def emit_gather_onehot(q, hh, tbl):
                    C = int(CQH[q, hh])
                    nix = int(nidx_stat[q, hh])
                    nj = len(jobs[(q, hh)])
                    g = poole.tile([P, CMAXG * HID], bf16, tag=f"g{hh}",
                                   name=f"g{hh}")
                    soh = poole.tile([P, W * KSLOT], bf16, tag=f"soh{hh}",
                                     name=f"soh{hh}")
                    # SWDGE ring caps one gather at 1024 descriptors
                    c0 = 0
                    while c0 < C:
                        cs_ = min(8, C - c0)
                        ni = cs_ * P
                        nc.gpsimd.dma_gather(
                            out_ap=g[:, c0 * HID:(c0 + cs_) * HID]
                            .rearrange("p (c f) -> p c f", f=HID),
                            in_ap=tbl[:, :],
                            idxs_ap=idx_sb[:, idx_off[q, hh] + c0 * 8:
                                           idx_off[q, hh] + (c0 + cs_) * 8],
                            num_idxs=ni,
                            num_idxs_reg=ni,
                            elem_size=HID,
                            queue_num=(2 * q + hh + c0 // 8) % 4,
                        )
                        c0 += cs_
                    if nj:
                        nc.vector.tensor_tensor(
                            out=soh[:, :W * nj].rearrange(
                                "p (t k) -> p t k", k=nj),
                            in0=iotk[:].rearrange(
                                "p (t k) -> p t k", k=KSLOT)[:, :, :nj],
                            in1=cmw_sb[:, slot_off[q, hh]:
                                       slot_off[q, hh] + nj]
                            .unsqueeze(1).to_broadcast([P, W, nj]),
                            op=Alu.is_equal)
                    return g, soh

                def emit_jobs(q, hh, g, soh, pagg, nn):
                    nj = len(jobs[(q, hh)])
                    nc.tensor.matmul(pagg[:, :nn], zerosb[:],
                                     identb4[:, :nn], start=True,
                                     stop=(nj == 0))
                    if nj == 0:
                        return
                    soh3 = soh[:, :W * nj].rearrange("p (t k) -> p t k", k=nj)
                    for sl, (ch, tb, o) in enumerate(jobs[(q, hh)]):
                        ob = tb * P + o
                        nc.tensor.matmul(
                            pagg[:, ob:ob + W],
                            g[:, ch * HID:(ch + 1) * HID],
                            soh3[:, :, sl], start=False, stop=False)
                    nc.tensor.matmul(pagg[:, :nn], zerosb[:],
                                     identb4[:, :nn], start=False, stop=True)

                # phase A: gather from table_a, spill partial aggregates
                for q in range(NQ):
                    ntile = min(G, T_C - q * G)
                    nn = ntile * P
                    qb = q * G * P
                    g, soh = emit_gather_onehot(q, 0, table_a)
                    pagg = epp.tile([P, G * P], f32, tag="pagg")
                    emit_jobs(q, 0, g, soh, pagg, nn)
                    nc.scalar.activation(spill[:, qb:qb + nn], pagg[:, :nn],
                                         AF.Identity)

                # phase B: gather from table_b, combine + output layers
                for q in range(NQ):
                    ntile = min(G, T_C - q * G)
                    nn = ntile * P
                    qb = q * G * P
                    g, soh = emit_gather_onehot(q, 1, table_b)
                    pagg = epp.tile([P, G * P], f32, tag="pagg")
                    emit_jobs(q, 1, g, soh, pagg, nn)
                    paggb = pool.tile([P, G * P], bf16, tag="paggb")
                    nc.scalar.activation(paggb[:, :nn], pagg[:, :nn],
                                         AF.Identity)
                    u1 = pool.tile([P, G * P], bf16, tag="u1")
                    nc.vector.tensor_add(u1[:, :nn], paggb[:, :nn],
                                         spill[:, qb:qb + nn])
                    m2a = pool.tile([P, G * P], bf16, tag="m2a")
                    nc.vector.tensor_mul(m2a[:, :nn], u1[:, :nn],
                                         dinvbb[:, qb:qb + nn])
                    m2b = pool.tile([P, G * P], bf16, tag="m2b")
                    nc.vector.tensor_add(m2b[:, :nn], m2a[:, :nn],
                                         selfd[:, qb:qb + nn])
                    pu = epp2.tile([P, G * P], f32, tag="pu")
                    nc.tensor.matmul(pu[:, :nn], wb[:, WUPD:WUPD + P],
                                     m2b[:, :nn], start=True, stop=True)
                    lu = pool.tile([P, G * P], bf16, tag="lu")
                    emit_lrelu(lu[:, :nn], pu[:, :nn], bias[:, 4:5], pool, nn)
                    po = epp2.tile([OUT_DIM, G * P], f32, tag="po")
                    nc.tensor.matmul(po[:, :nn], wb[:, WCLS:WCLS + OUT_DIM],
                                     lu[:, :nn], start=True, stop=True)
                    ot = pool.tile([OUT_DIM, G * P], f32, tag="ot")
                    nc.scalar.activation(ot[:, :nn], po[:, :nn], AF.Identity,
                                         bias=bias[0:OUT_DIM, 5:6])
                    nc.sync.dma_start(d_out[:, qb:qb + nn], ot[:, :nn])

    nc.compile()
    return nc


def _run_spmd_presharded(nc, in_maps, n_cores=N_CORES):
    """Run a compiled Bass program on n_cores via PJRT with host-side
    pre-sharded inputs (avoids XLA reshard programs on big arrays)."""
    import jax
    import concourse.mybir as mybir
    from concourse import bass2jax
    from jax.sharding import Mesh, PartitionSpec, NamedSharding
    from jax.experimental.shard_map import shard_map

    bass2jax.install_neuronx_cc_hook()
    partition_name = nc.partition_id_tensor.name if nc.partition_id_tensor else None
    in_names, out_names, out_avals, zero_outs = [], [], [], []
    for alloc in nc.m.functions[0].allocations:
        if not isinstance(alloc, mybir.MemoryLocationSet):
            continue
        name = alloc.memorylocations[0].name
        if alloc.kind == "ExternalInput":
            if name != partition_name:
                in_names.append(name)
        elif alloc.kind == "ExternalOutput":
            out_names.append(name)
            shape = tuple(alloc.tensor_shape)
            dtype = mybir.dt.np(alloc.dtype)
            out_avals.append(jax.core.ShapedArray(shape, dtype))
            zero_outs.append(np.zeros(shape, dtype))
    n_params = len(in_names)
    in_names_all = list(in_names) + out_names
    if partition_name is not None:
        in_names_all.append(partition_name)

    def _body(*args):
        operands = list(args)
        if partition_name is not None:
            operands.append(bass2jax.partition_id_tensor())
        outs = bass2jax._bass_exec_p.bind(
            *operands,
            out_avals=tuple(out_avals),
            in_names=tuple(in_names_all),
            out_names=tuple(out_names),
            lowering_input_output_aliases=(),
            sim_require_finite=True,
            sim_require_nnan=True,
            nc=nc,
        )
        return tuple(outs)

    devices = jax.devices()[:n_cores]
    mesh = Mesh(np.asarray(devices), ("core",))
    spec = PartitionSpec("core")
    n_outs = len(out_avals)
    sharded = jax.jit(
        shard_map(_body, mesh=mesh, in_specs=(spec,) * (n_params + n_outs),
                  out_specs=(spec,) * n_outs, check_rep=False),
        keep_unused=True,
    )
    sh = NamedSharding(mesh, spec)

    def put(per_core_arrays):
        a0 = np.asarray(per_core_arrays[0])
        gshape = (n_cores * a0.shape[0],) + a0.shape[1:]
        shards = [jax.device_put(np.ascontiguousarray(per_core_arrays[c]),
                                 devices[c]) for c in range(n_cores)]
        return jax.make_array_from_single_device_arrays(gshape, sh, shards)

    args = [put([m[name] for m in in_maps]) for name in in_names]
    args += [put([z] * n_cores) for z in zero_outs]
    out_arrs = sharded(*args)
    jax.block_until_ready(out_arrs)
    return [
        {name: np.asarray(out_arrs[i]).reshape(n_cores, *out_avals[i].shape)[c]
         for i, name in enumerate(out_names)}
        for c in range(n_cores)
    ]


def kernel(x, edge_index, W_in, b_in, W_nor, b_nor, W_abnor, b_abnor,
           W_att, b_att, v_att, W_upd, b_upd, W_cls, b_cls):
    import ml_dtypes
    bf = ml_dtypes.bfloat16

    x = np.asarray(x, np.float32)
    n = x.shape[0]
    meta = _host_plan(x, edge_index)
    NSH = meta["NSH"]
    nc = _build_program(meta, with_collective=True)

    Wnor_p = np.zeros((P, P), np.float32)
    Wnor_p[:HALF] = np.asarray(W_nor, np.float32)
    Wab_p = np.zeros((P, P), np.float32)
    Wab_p[HALF:] = np.asarray(W_abnor, np.float32)
    b_nor_ = np.asarray(b_nor, np.float32)
    b_ab_ = np.asarray(b_abnor, np.float32)
    W_st = Wnor_p + Wab_p
    b_st = b_nor_ + b_ab_
    wbk = np.zeros((P, 6 * P + 3), np.float32)
    wbk[:, 0:P] = np.asarray(W_in, np.float32)[:P, :]
    wbk[:, P:2 * P] = np.asarray(W_in, np.float32)[P:, :]
    wbk[:, 2 * P:3 * P] = 0.5 * W_st                       # r = 0.5(xn+xa)
    wbk[:, 3 * P:4 * P] = 0.5 * (Wnor_p - Wab_p)           # ddh = 0.5(xn-xa)
    wbk[:, 4 * P:5 * P] = W_st @ np.asarray(W_att, np.float32)
    wbk[:, 5 * P:6 * P] = np.asarray(W_upd, np.float32)
    wbk[:, 6 * P:6 * P + 2] = np.asarray(W_cls, np.float32)
    wbk[:, 6 * P + 2] = np.asarray(v_att, np.float32).reshape(-1) * 0.5

    bias = np.zeros((P, 6), np.float32)
    bias[:, 0] = np.asarray(b_in, np.float32)
    bias[:, 1] = 0.5 * b_st
    bias[:, 2] = 0.5 * (b_nor_ - b_ab_)
    bias[:, 3] = (b_st @ np.asarray(W_att, np.float32)
                  + np.asarray(b_att, np.float32))
    bias[:, 4] = np.asarray(b_upd, np.float32)
    bias[:OUT_DIM, 5] = np.asarray(b_cls, np.float32)

    shared = {
        "wb": wbk.astype(bf),
        "bias": bias,
        "iota_tk": meta["iota_tk"].astype(bf),
        "b_ddh_r": (0.5 * (b_nor_ - b_ab_)).reshape(1, P).astype(bf),
    }
    x_t = meta["x_t"]
    in_maps = []
    for c in range(N_CORES):
        xc = np.concatenate([x_t[:P, c * NSH:(c + 1) * NSH],
                             x_t[P:, c * NSH:(c + 1) * NSH]], axis=1)
        in_maps.append({
            **shared,
            "xq": np.ascontiguousarray(xc).astype(bf),
            "deg_ct": meta["deg_ct"][c],
            "deg_r": meta["deg_r"][c],
            "idx": meta["idx_all"][c],
            "cmw": meta["cm_all"][c].astype(bf),
        })

    results = _run_spmd_presharded(nc, in_maps)
    out_full = np.empty((meta["NP"], OUT_DIM), np.float32)
    for c in range(N_CORES):
        out_full[c * NSH:(c + 1) * NSH] = results[c]["outp"].T
    return np.ascontiguousarray(out_full[:n])


# revision 29
# speedup vs baseline: 1.5327x; 1.0171x over previous
"""CGNN message-passing kernel for Trainium2, 8 NeuronCores.

Strategy (v3):
  - Algebraic reduction: the attention gate depends only on the source node,
    so the edge computation collapses to aggr = dinv (x) (A @ m') with
    m'[j] = dinv_j*(alpha_j*xn_j + (1-alpha_j)*xa_j) a per-node table.
  - Contiguous node sharding: core c owns nodes [c*NSH, (c+1)*NSH) for both
    phases; self-loop terms are added from SBUF (no gathering of self rows).
  - Node phase is fully feature-major and slab-batched: the alpha logit is a
    [1, n] row matmul, alpha*dinv and dinv broadcasts are outer-product
    matmuls, the message mix is three slab-wide DVE ops; only the final
    node-major table rows need a PE transpose per tile. Sigmoid is realized
    as 0.5*tanh(q/2)+0.5 (v_att pre-halved) so Lrelu/Tanh/Identity share one
    activation table.
  - Edge phase: edges sorted by (4-tile quad, table half, target); one SWDGE
    dma_gather per (quad, half) with exact max-count index streams. The
    scatter-add is PSUM-accumulated matmuls over a narrow one-hot: chunks of
    128 target-sorted edges span only ~8 targets, so each matmul writes a
    32-column window of the packed [128, 512] quad PSUM. Window offsets are
    compile-time (union over cores, computed per run); per-core variability
    lives in the cm data (sentinels mask pad slots). The one-hot is built in
    [p, t, k] layout against a materialized iota so all DVE operands are
    packed 2-byte -> 2x DVE rate.
"""
import numpy as np

N_CORES = 8
P = 128
IN_DIM = 256
HID = 128
HALF = 64
OUT_DIM = 2
LRELU_SLOPE = 0.01
SLAB = 4                  # node-phase tiles per slab (<=512 cols)
G = 4                     # edge-phase tiles per quad
W = 32                    # one-hot window width
SENT = 200.0              # one-hot sentinel (never matches a window col)


def _wrap16(flat):
    """[n] int16 -> [128, ceil(n/16)] in the SWDGE 16-wrap + 8x replicated
    layout (flat position i lands at row i%16, col i//16)."""
    n = len(flat)
    cols = -(-n // 16)
    pad = np.zeros(cols * 16, np.int16)
    pad[:n] = flat
    blk = pad.reshape(cols, 16).T          # [16, cols]
    return np.tile(blk, (8, 1))            # [128, cols]


def _host_plan(x, edge_index):
    n = x.shape[0]
    NP = ((n + 1023) // 1024) * 1024       # 50176
    NSH = NP // N_CORES                    # 6272
    T_C = NSH // P                         # 49
    _min_a = -(-max(0, NSH - 4095) // P)   # int16: 8*SPB <= 32767
    SPL_T = max(SLAB, -(-_min_a // SLAB) * SLAB)   # 20 tiles -> table A
    SPA = SPL_T * P                        # 3072 rows/core in table A
    SPB = NSH - SPA                        # 3200 rows/core in table B
    NQ = -(-T_C // G)                      # 13 quads (last has 1 tile)

    ei = np.asarray(edge_index)
    row = ei[0].astype(np.int64)
    col = ei[1].astype(np.int64)

    deg = np.bincount(col, minlength=NP).astype(np.float32)
    deg[:n] += 1.0                         # self loops
    deg[n:] = 1.0                          # pad nodes stay finite

    core_e = col // NSH
    r_e = row % NSH
    half_e = (r_e >= SPA).astype(np.int64)
    src_c = row // NSH
    src_remap = np.where(half_e == 0, src_c * SPA + r_e,
                         src_c * SPB + (r_e - SPA))
    lt_e = (col % NSH) // P                # local tile 0..48
    q_e = lt_e // G

    # per (core, quad, half) edge counts -> static stream sizes
    cnt = np.zeros((N_CORES, NQ, 2), np.int64)
    np.add.at(cnt, (core_e, q_e, half_e), 1)
    nmax = cnt.max(axis=0)                 # [NQ, 2]
    nidx_stat = ((nmax + 127) // 128) * 128  # num_idxs per gather
    CQH = -(-nidx_stat // P)               # gather chunks
    idx_cols = nidx_stat // 16
    idx_off = np.zeros((NQ, 2), np.int64)
    io = 0
    for q in range(NQ):
        for h in range(2):
            idx_off[q, h] = io
            io += idx_cols[q, h]
    IDXC = int(io)
    CMAXG = int(CQH.max())

    order = np.lexsort((col, half_e, q_e, core_e))
    rms = src_remap[order]
    cs = col[order]
    bounds = np.concatenate([[0], np.cumsum(cnt.reshape(-1))])

    # per-core streams: sources, tile-in-quad, local col
    streams = {}
    for c in range(N_CORES):
        for q in range(NQ):
            for h in range(2):
                bi = (c * NQ + q) * 2 + h
                b0, b1 = bounds[bi], bounds[bi + 1]
                streams[(c, q, h)] = (rms[b0:b1],
                                      (cs[b0:b1] % NSH) // P - q * G,
                                      (cs[b0:b1] % P))

    # jobs: per (q, h): [(chunk, tile-in-quad, window offset)], windows from
    # the union of target cols over cores (exact, computed per run)
    jobs = {}
    slot_off = np.zeros((NQ, 2), np.int64)
    so = 0
    KSLOT = 0
    for q in range(NQ):
        for h in range(2):
            slot_off[q, h] = so
            jl = []
            ucols = {}
            for c in range(N_CORES):
                _, tbs, lcs = streams[(c, q, h)]
                pos = np.arange(len(tbs))
                key = (pos // P) * G + tbs
                for kk in np.unique(key):
                    m = key == kk
                    s = ucols.setdefault(int(kk), set())
                    s.update(lcs[m].tolist())
            for kk in sorted(ucols):
                ch, tb = kk // G, kk % G
                colss = sorted(ucols[kk])
                i = 0
                while i < len(colss):
                    o = min(colss[i], P - W)
                    j = i
                    while j < len(colss) and colss[j] < o + W:
                        j += 1
                    jl.append((ch, tb, o))
                    i = j
            jobs[(q, h)] = jl
            so += len(jl)
            KSLOT = max(KSLOT, len(jl))
    NSLOT = int(so)

    idx_all = np.zeros((N_CORES, P, IDXC), np.int16)
    cm_all = np.full((N_CORES, P, NSLOT), SENT, np.float32)
    for c in range(N_CORES):
        for q in range(NQ):
            for h in range(2):
                srcs, tbs, lcs = streams[(c, q, h)]
                nqh = int(nidx_stat[q, h])
                stream = np.zeros(nqh, np.int64)
                stream[:len(srcs)] = srcs
                idx_all[c, :, idx_off[q, h]:idx_off[q, h] + idx_cols[q, h]] = \
                    _wrap16(stream.astype(np.int16))
                pos = np.arange(len(tbs))
                assigned = np.zeros(len(tbs), bool)
                for sl, (ch, tb, o) in enumerate(jobs[(q, h)]):
                    m = ((pos // P == ch) & (tbs == tb) & (lcs >= o)
                         & (lcs < o + W) & ~assigned)
                    if m.any():
                        assigned |= m
                        cm_all[c, pos[m] % P, slot_off[q, h] + sl] = lcs[m] - o

    x_t = np.zeros((IN_DIM, NP), np.float32)
    x_t[:, :n] = np.asarray(x, np.float32).T

    deg_ct = deg.reshape(N_CORES, T_C, P).transpose(0, 2, 1)  # [c, 128, T_C]
    deg_r = deg.reshape(N_CORES, 1, NSH)

    # iota for the windowed one-hot: value t at (p, t*KSLOT + k)
    iota_tk = np.tile(np.repeat(np.arange(W, dtype=np.float32), KSLOT)[None, :],
                      (P, 1))

    return dict(NP=NP, NSH=NSH, T_C=T_C, SPL_T=SPL_T, SPA=SPA, SPB=SPB,
                NQ=NQ,
                CQH=CQH, nidx_stat=nidx_stat, idx_cols=idx_cols,
                idx_off=idx_off, slot_off=slot_off, jobs=jobs,
                IDXC=IDXC, NSLOT=NSLOT, CMAXG=CMAXG, KSLOT=KSLOT,
                idx_all=idx_all, cm_all=cm_all, x_t=x_t, deg_ct=deg_ct,
                deg_r=deg_r, iota_tk=iota_tk)


def _build_program(meta, with_collective=True, act_lrelu=True):
    import concourse.bass as bass
    import concourse.bacc as bacc
    import concourse.mybir as mybir
    import concourse.tile as tile
    from concourse.masks import make_identity

    f32 = mybir.dt.float32
    bf16 = mybir.dt.bfloat16
    i16 = mybir.dt.int16
    AF = mybir.ActivationFunctionType
    Alu = mybir.AluOpType

    NSH, T_C, NP = meta["NSH"], meta["T_C"], meta["NP"]
    SPL_T, SPA, SPB = meta["SPL_T"], meta["SPA"], meta["SPB"]
    NQ = meta["NQ"]
    CQH = meta["CQH"]
    nidx_stat = meta["nidx_stat"]
    idx_cols = meta["idx_cols"]
    idx_off = meta["idx_off"]
    slot_off = meta["slot_off"]
    jobs = meta["jobs"]
    IDXC, NSLOT, CMAXG, KSLOT = (meta["IDXC"], meta["NSLOT"], meta["CMAXG"],
                                 meta["KSLOT"])

    nc = bacc.Bacc("TRN2", target_bir_lowering=False, debug=False,
                   num_swdge_queues=4)
    table_a = nc.dram_tensor("cc_table_a", [N_CORES * SPA, HID], bf16,
                             addr_space="Shared")
    table_b = nc.dram_tensor("cc_table_b", [N_CORES * SPB, HID], bf16,
                             addr_space="Shared")

    d_xq = nc.dram_tensor("xq", [P, 2 * NSH], bf16, kind="ExternalInput")
    d_wb = nc.dram_tensor("wb", [P, 6 * P + 3], bf16, kind="ExternalInput")
    d_bias = nc.dram_tensor("bias", [P, 6], f32, kind="ExternalInput")
    d_degct = nc.dram_tensor("deg_ct", [P, T_C], f32, kind="ExternalInput")
    d_degr = nc.dram_tensor("deg_r", [1, NSH], f32, kind="ExternalInput")
    d_idx = nc.dram_tensor("idx", [P, IDXC], i16, kind="ExternalInput")
    d_cmw = nc.dram_tensor("cmw", [P, NSLOT], bf16, kind="ExternalInput")
    d_iotk = nc.dram_tensor("iota_tk", [P, W * KSLOT], bf16,
                            kind="ExternalInput")
    d_bddh = nc.dram_tensor("b_ddh_r", [1, P], bf16, kind="ExternalInput")
    d_out = nc.dram_tensor("outp", [OUT_DIM, NSH], f32, kind="ExternalOutput")

    WA, WB, WNOR, WAB, WATT, WUPD, WCLS, VH = (0, P, 2 * P, 3 * P, 4 * P,
                                               5 * P, 6 * P, 6 * P + 2)

    def emit_lrelu(out_ap, psum_ap, bias_ap, tmp_pool, nncols):
        if act_lrelu:
            nc.scalar.activation(out_ap, psum_ap, AF.Lrelu, bias=bias_ap,
                                 alpha=LRELU_SLOPE)
        else:
            nc.scalar.activation(out_ap, psum_ap, AF.Identity, bias=bias_ap)
            tl = tmp_pool.tile([P, 512], bf16, tag="lrtmp", name="lrtmp")
            nc.vector.tensor_scalar(out=tl[:, :nncols], in0=out_ap,
                                    scalar1=LRELU_SLOPE, scalar2=None,
                                    op0=Alu.mult)
            nc.vector.tensor_tensor(out=out_ap, in0=out_ap,
                                    in1=tl[:, :nncols], op=Alu.max)

    with tile.TileContext(nc) as tc:
        with (
            tc.tile_pool(name="const", bufs=1) as cpool,
            tc.tile_pool(name="sbuf", bufs=2) as pool,
            tc.tile_pool(name="sbe", bufs=2) as poole,
            tc.tile_pool(name="dram", bufs=1, space="DRAM") as dpool,
        ):
            # ---------- constants ----------
            wb = cpool.tile([P, 6 * P + 3], bf16)
            nc.sync.dma_start(wb[:], d_wb[:])
            bias = cpool.tile([P, 6], f32)
            nc.sync.dma_start(bias[:], d_bias[:])
            iotk = cpool.tile([P, W * KSLOT], bf16)
            idx_sb = cpool.tile([P, IDXC], i16)
            cmw_sb = cpool.tile([P, NSLOT], bf16)

            identb = cpool.tile([P, P], bf16)
            make_identity(nc, identb[:])
            identb4 = cpool.tile([P, G * P], bf16)
            for j in range(G):
                nc.vector.tensor_scalar(out=identb4[:, j * P:(j + 1) * P],
                                        in0=identb[:], scalar1=1.0,
                                        scalar2=None, op0=Alu.mult)
            zerosb = cpool.tile([P, P], bf16)
            nc.vector.memset(zerosb[:], 0.0)
            ones1 = cpool.tile([1, P], bf16)
            nc.vector.memset(ones1[:], 1.0)
            onesr = cpool.tile([1, 512], bf16)
            nc.vector.memset(onesr[:], 1.0)
            bddh = cpool.tile([1, P], bf16)
            nc.sync.dma_start(bddh[:], d_bddh[:])

            dct = cpool.tile([P, T_C], f32)
            nc.sync.dma_start(dct[:], d_degct[:])
            nc.scalar.activation(dct[:], dct[:], AF.Sqrt)
            nc.vector.reciprocal(dct[:], dct[:])
            # flat bf16 dinv row [1, NSH]: PE-transpose dct, bounce via DRAM
            dctb = cpool.tile([P, P], bf16)
            nc.vector.memset(dctb[:], 1.0)
            nc.scalar.activation(dctb[:, :T_C], dct[:], AF.Identity)
            dinvr_b = cpool.tile([1, NSH], bf16)
            scr = dpool.tile([P, P], bf16)
            with tc.tile_pool(name="tps", bufs=1, space="PSUM") as tpp:
                ptc = tpp.tile([P, P], bf16)
                nc.tensor.transpose(ptc[:], dctb[:], identb[:])
                dctT = cpool.tile([P, P], bf16)
                nc.scalar.activation(dctT[:], ptc[:], AF.Identity)
                nc.sync.dma_start(scr[:], dctT[:])
                nc.sync.dma_start(
                    dinvr_b[:],
                    scr[:].rearrange("t p -> (t p)")[:NSH].unsqueeze(0))

            # persistent node-phase products (feature-major)
            selfd = cpool.tile([P, NSH], bf16)    # dinv (x) m' (self term)
            dinvbb = cpool.tile([P, NSH], bf16)   # dinv[t] bcast per column

            shard_a = dpool.tile([SPA, HID], bf16)
            shard_b = dpool.tile([SPB, HID], bf16)
            shard3a = shard_a[:].rearrange("(t p) f -> p t f", p=P)
            shard3b = shard_b[:].rearrange("(t p) f -> p t f", p=P)
            # spill for phase-A partial aggregates
            spill = cpool.tile([P, G * P * NQ], bf16)

            # ---------- node phase ----------
            with tc.tile_pool(name="npsum", bufs=4, space="PSUM") as npp, \
                 tc.tile_pool(name="nppal", bufs=2, space="PSUM") as nppal, \
                 tc.tile_pool(name="nptr", bufs=2, space="PSUM") as nptr:
                t0 = 0
                while t0 < T_C:
                    nt = min(SLAB, T_C - t0)
                    nn = nt * P
                    nb = t0 * P
                    pdb = npp.tile([P, 512], f32, tag="mm")
                    nc.tensor.matmul(pdb[:, :nn], ones1[:],
                                     dinvr_b[:, nb:nb + nn], start=True,
                                     stop=True)
                    nc.scalar.activation(dinvbb[:, nb:nb + nn], pdb[:, :nn],
                                         AF.Identity)
                    t0 += nt
                slabs = []
                t0 = 0
                while t0 < T_C:
                    nt = min(SLAB, T_C - t0)
                    slabs.append((t0, nt))
                    t0 += nt
                SPL_SLAB = SPL_T // SLAB
                state = {}

                def stage1(i):
                    t0, nt = slabs[i]
                    nn = nt * P
                    nb = t0 * P
                    xsl = pool.tile([P, 2 * 512], bf16, tag="xsl",
                                    name="xsl")
                    nc.sync.dma_start(
                        xsl[:, :2 * nn].rearrange("p (a n) -> p a n", a=2),
                        d_xq[:].rearrange("p (a n) -> p a n",
                                          a=2)[:, :, nb:nb + nn])
                    ph = npp.tile([P, 512], f32, tag="mm", name="ph")
                    nc.tensor.matmul(ph[:, :nn], wb[:, WA:WA + P],
                                     xsl[:, :nn], start=True, stop=False)
                    nc.tensor.matmul(ph[:, :nn], wb[:, WB:WB + P],
                                     xsl[:, nn:2 * nn],
                                     start=False, stop=True)
                    h = pool.tile([P, 512], bf16, tag="h", name="h")
                    emit_lrelu(h[:, :nn], ph[:, :nn], bias[:, 0:1], pool, nn)
                    pr = npp.tile([P, 512], f32, tag="mm", name="pr")
                    nc.tensor.matmul(pr[:, :nn], wb[:, WNOR:WNOR + P],
                                     h[:, :nn], start=True, stop=True)
                    pd = npp.tile([P, 512], f32, tag="mm", name="pd")
                    nc.tensor.matmul(pd[:, :nn], wb[:, WAB:WAB + P],
                                     h[:, :nn], start=True, stop=False)
                    nc.tensor.matmul(pd[:, :nn], bddh[:], onesr[:, :nn],
                                     start=False, stop=True)
                    patt = npp.tile([P, 512], f32, tag="mm", name="patt")
                    nc.tensor.matmul(patt[:, :nn], wb[:, WATT:WATT + P],
                                     h[:, :nn], start=True, stop=True)
                    rr = pool.tile([P, 512], bf16, tag="rr", name="rr")
                    nc.vector.tensor_scalar(out=rr[:, :nn], in0=pr[:, :nn],
                                            scalar1=bias[:, 1:2],
                                            scalar2=None, op0=Alu.add)
                    hatt = pool.tile([P, 512], bf16, tag="hatt", name="hatt")
                    nc.scalar.activation(hatt[:, :nn], patt[:, :nn], AF.Tanh,
                                         bias=bias[:, 3:4])
                    pal = nppal.tile([1, 512], f32, tag="pal", name="pal")
                    nc.tensor.matmul(pal[:, :nn], wb[:, VH:VH + 1],
                                     hatt[:, :nn], start=True, stop=True)
                    sig = pool.tile([1, 512], bf16, tag="sig", name="sig")
                    nc.scalar.activation(sig[:, :nn], pal[:, :nn], AF.Tanh)
                    state[i] = (rr, pd, sig)

                def stage2(i):
                    t0, nt = slabs[i]
                    nn = nt * P
                    nn2 = (nt // 2) * P if nt > 1 else nn
                    nb = t0 * P
                    rr, pd, sig = state.pop(i)
                    psg = npp.tile([P, 512], f32, tag="mm", name="psg")
                    nc.tensor.matmul(psg[:, :nn], ones1[:], sig[:, :nn],
                                     start=True, stop=True)
                    sgb = pool.tile([P, 512], f32, tag="sgb", name="sgb")
                    nc.scalar.activation(sgb[:, :nn], psg[:, :nn],
                                         AF.Identity)
                    s2 = pool.tile([P, 512], bf16, tag="s2", name="s2")
                    nc.vector.tensor_mul(s2[:, :nn], pd[:, :nn],
                                         sgb[:, :nn])
                    ms = pool.tile([P, 512], bf16, tag="ms", name="ms")
                    nc.vector.tensor_add(ms[:, :nn], rr[:, :nn], s2[:, :nn])
                    mfs = pool.tile([P, 512], bf16, tag="mfs", name="mfs")
                    nc.vector.tensor_mul(mfs[:, :nn], ms[:, :nn],
                                         dinvbb[:, nb:nb + nn])
                    nc.vector.tensor_mul(selfd[:, nb:nb + nn], mfs[:, :nn],
                                         dinvbb[:, nb:nb + nn])
                    ptm = nptr.tile([P, 512], bf16, tag="tr", name="tr")
                    for j in range(nt):
                        nc.tensor.transpose(
                            ptm[:, j * P:(j + 1) * P],
                            mfs[:, j * P:(j + 1) * P], identb[:])
                    mrow = pool.tile([P, 512], bf16, tag="mrow", name="mrow")
                    nc.scalar.activation(mrow[:, :nn2], ptm[:, :nn2],
                                         AF.Identity)
                    if nn > nn2:
                        nc.vector.tensor_scalar(out=mrow[:, nn2:nn],
                                                in0=ptm[:, nn2:nn],
                                                scalar1=1.0, scalar2=None,
                                                op0=Alu.mult)
                    if t0 < SPL_T:
                        nc.sync.dma_start(
                            shard3a[:, t0:t0 + nt, :],
                            mrow[:, :nn].rearrange("p (t f) -> p t f", f=P))
                    else:
                        nc.sync.dma_start(
                            shard3b[:, t0 - SPL_T:t0 - SPL_T + nt, :],
                            mrow[:, :nn].rearrange("p (t f) -> p t f", f=P))

                for i in range(len(slabs)):
                    stage1(i)
                    if i == 1:
                        nc.sync.dma_start(iotk[:], d_iotk[:])
                        nc.sync.dma_start(idx_sb[:], d_idx[:])
                        nc.sync.dma_start(cmw_sb[:], d_cmw[:])
                    if i > 0:
                        stage2(i - 1)
                    if i == SPL_SLAB:
                        if with_collective:
                            nc.gpsimd.collective_compute(
                                "AllGather", mybir.AluOpType.bypass,
                                replica_groups=[list(range(N_CORES))],
                                ins=[shard_a.opt()], outs=[table_a[:]])
                        else:
                            nc.sync.dma_start(table_a[:SPA, :], shard_a[:])
                stage2(len(slabs) - 1)

            # ---------- replicate table (half B) ----------
            if with_collective:
                nc.gpsimd.collective_compute(
                    "AllGather",
                    mybir.AluOpType.bypass,
                    replica_groups=[list(range(N_CORES))],
                    ins=[shard_b.opt()],
                    outs=[table_b[:]],
                )
            else:
                nc.sync.dma_start(table_b[:SPB, :], shard_b[:])

            # ---------- edge phase ----------
            with tc.tile_pool(name="epsum", bufs=3, space="PSUM") as epp, \
                 tc.tile_pool(name="eps2", bufs=2, space="PSUM") as epp2:

                def emit_gather_onehot(q, hh, tbl):
                    C = int(CQH[q, hh])
                    nix = int(nidx_stat[q, hh])
                    nj = len(jobs[(q, hh)])
                    g = poole.tile([P, CMAXG * HID], bf16, tag=f"g{hh}",
                                   name=f"g{hh}")
                    soh = poole.tile([P, W * KSLOT], bf16, tag=f"soh{hh}",
                                     name=f"soh{hh}")
                    # SWDGE ring caps one gather at 1024 descriptors
                    c0 = 0
                    while c0 < C:
                        cs_ = min(8, C - c0)
                        ni = cs_ * P
                        nc.gpsimd.dma_gather(
                            out_ap=g[:, c0 * HID:(c0 + cs_) * HID]
                            .rearrange("p (c f) -> p c f", f=HID),
                            in_ap=tbl[:, :],
                            idxs_ap=idx_sb[:, idx_off[q, hh] + c0 * 8:
                                           idx_off[q, hh] + (c0 + cs_) * 8],
                            num_idxs=ni,
                            num_idxs_reg=ni,
                            elem_size=HID,
                            queue_num=(2 * q + hh + c0 // 8) % 4,
                        )
                        c0 += cs_
                    if nj:
                        nc.vector.tensor_tensor(
                            out=soh[:, :W * nj].rearrange(
                                "p (t k) -> p t k", k=nj),
                            in0=iotk[:].rearrange(
                                "p (t k) -> p t k", k=KSLOT)[:, :, :nj],
                            in1=cmw_sb[:, slot_off[q, hh]:
                                       slot_off[q, hh] + nj]
                            .unsqueeze(1).to_broadcast([P, W, nj]),
                            op=Alu.is_equal)
                    return g, soh

                def emit_jobs(q, hh, g, soh, pagg, nn):
                    nj = len(jobs[(q, hh)])
                    nc.tensor.matmul(pagg[:, :nn], zerosb[:],
                                     identb4[:, :nn], start=True,
                                     stop=(nj == 0))
                    if nj == 0:
                        return
                    soh3 = soh[:, :W * nj].rearrange("p (t k) -> p t k", k=nj)
                    for sl, (ch, tb, o) in enumerate(jobs[(q, hh)]):
                        ob = tb * P + o
                        nc.tensor.matmul(
                            pagg[:, ob:ob + W],
                            g[:, ch * HID:(ch + 1) * HID],
                            soh3[:, :, sl], start=False, stop=False)
                    nc.tensor.matmul(pagg[:, :nn], zerosb[:],
                                     identb4[:, :nn], start=False, stop=True)

                # phase A: gather from table_a, spill partial aggregates
                for q in range(NQ):
                    ntile = min(G, T_C - q * G)
                    nn = ntile * P
                    qb = q * G * P
                    g, soh = emit_gather_onehot(q, 0, table_a)
                    pagg = epp.tile([P, G * P], f32, tag="pagg")
                    emit_jobs(q, 0, g, soh, pagg, nn)
                    nc.scalar.activation(spill[:, qb:qb + nn], pagg[:, :nn],
                                         AF.Identity)

                # phase B: gather from table_b, combine + output layers
                for q in range(NQ):
                    ntile = min(G, T_C - q * G)
                    nn = ntile * P
                    qb = q * G * P
                    g, soh = emit_gather_onehot(q, 1, table_b)
                    pagg = epp.tile([P, G * P], f32, tag="pagg")
                    emit_jobs(q, 1, g, soh, pagg, nn)
                    paggb = pool.tile([P, G * P], bf16, tag="paggb")
                    nc.scalar.activation(paggb[:, :nn], pagg[:, :nn],
                                         AF.Identity)
                    u1 = pool.tile([P, G * P], bf16, tag="u1")
                    nc.vector.tensor_add(u1[:, :nn], paggb[:, :nn],
                                         spill[:, qb:qb + nn])
                    m2a = pool.tile([P, G * P], bf16, tag="m2a")
                    nc.vector.tensor_mul(m2a[:, :nn], u1[:, :nn],
                                         dinvbb[:, qb:qb + nn])
                    m2b = pool.tile([P, G * P], bf16, tag="m2b")
                    nc.vector.tensor_add(m2b[:, :nn], m2a[:, :nn],
                                         selfd[:, qb:qb + nn])
                    pu = epp2.tile([P, G * P], f32, tag="pu")
                    nc.tensor.matmul(pu[:, :nn], wb[:, WUPD:WUPD + P],
                                     m2b[:, :nn], start=True, stop=True)
                    lu = pool.tile([P, G * P], bf16, tag="lu")
                    emit_lrelu(lu[:, :nn], pu[:, :nn], bias[:, 4:5], pool, nn)
                    po = epp2.tile([OUT_DIM, G * P], f32, tag="po")
                    nc.tensor.matmul(po[:, :nn], wb[:, WCLS:WCLS + OUT_DIM],
                                     lu[:, :nn], start=True, stop=True)
                    ot = pool.tile([OUT_DIM, G * P], f32, tag="ot")
                    nc.scalar.activation(ot[:, :nn], po[:, :nn], AF.Identity,
                                         bias=bias[0:OUT_DIM, 5:6])
                    nc.sync.dma_start(d_out[:, qb:qb + nn], ot[:, :nn])

    nc.compile()
    return nc


def _run_spmd_presharded(nc, in_maps, n_cores=N_CORES):
    """Run a compiled Bass program on n_cores via PJRT with host-side
    pre-sharded inputs (avoids XLA reshard programs on big arrays)."""
    import jax
    import concourse.mybir as mybir
    from concourse import bass2jax
    from jax.sharding import Mesh, PartitionSpec, NamedSharding
    from jax.experimental.shard_map import shard_map

    bass2jax.install_neuronx_cc_hook()
    partition_name = nc.partition_id_tensor.name if nc.partition_id_tensor else None
    in_names, out_names, out_avals, zero_outs = [], [], [], []
    for alloc in nc.m.functions[0].allocations:
        if not isinstance(alloc, mybir.MemoryLocationSet):
            continue
        name = alloc.memorylocations[0].name
        if alloc.kind == "ExternalInput":
            if name != partition_name:
                in_names.append(name)
        elif alloc.kind == "ExternalOutput":
            out_names.append(name)
            shape = tuple(alloc.tensor_shape)
            dtype = mybir.dt.np(alloc.dtype)
            out_avals.append(jax.core.ShapedArray(shape, dtype))
            zero_outs.append(np.zeros(shape, dtype))
    n_params = len(in_names)
    in_names_all = list(in_names) + out_names
    if partition_name is not None:
        in_names_all.append(partition_name)

    def _body(*args):
        operands = list(args)
        if partition_name is not None:
            operands.append(bass2jax.partition_id_tensor())
        outs = bass2jax._bass_exec_p.bind(
            *operands,
            out_avals=tuple(out_avals),
            in_names=tuple(in_names_all),
            out_names=tuple(out_names),
            lowering_input_output_aliases=(),
            sim_require_finite=True,
            sim_require_nnan=True,
            nc=nc,
        )
        return tuple(outs)

    devices = jax.devices()[:n_cores]
    mesh = Mesh(np.asarray(devices), ("core",))
    spec = PartitionSpec("core")
    n_outs = len(out_avals)
    sharded = jax.jit(
        shard_map(_body, mesh=mesh, in_specs=(spec,) * (n_params + n_outs),
                  out_specs=(spec,) * n_outs, check_rep=False),
        keep_unused=True,
    )
    sh = NamedSharding(mesh, spec)

    def put(per_core_arrays):
        a0 = np.asarray(per_core_arrays[0])
        gshape = (n_cores * a0.shape[0],) + a0.shape[1:]
        shards = [jax.device_put(np.ascontiguousarray(per_core_arrays[c]),
                                 devices[c]) for c in range(n_cores)]
        return jax.make_array_from_single_device_arrays(gshape, sh, shards)

    args = [put([m[name] for m in in_maps]) for name in in_names]
    args += [put([z] * n_cores) for z in zero_outs]
    out_arrs = sharded(*args)
    jax.block_until_ready(out_arrs)
    return [
        {name: np.asarray(out_arrs[i]).reshape(n_cores, *out_avals[i].shape)[c]
         for i, name in enumerate(out_names)}
        for c in range(n_cores)
    ]


def kernel(x, edge_index, W_in, b_in, W_nor, b_nor, W_abnor, b_abnor,
           W_att, b_att, v_att, W_upd, b_upd, W_cls, b_cls):
    import ml_dtypes
    bf = ml_dtypes.bfloat16

    x = np.asarray(x, np.float32)
    n = x.shape[0]
    meta = _host_plan(x, edge_index)
    NSH = meta["NSH"]
    nc = _build_program(meta, with_collective=True)

    Wnor_p = np.zeros((P, P), np.float32)
    Wnor_p[:HALF] = np.asarray(W_nor, np.float32)
    Wab_p = np.zeros((P, P), np.float32)
    Wab_p[HALF:] = np.asarray(W_abnor, np.float32)
    b_nor_ = np.asarray(b_nor, np.float32)
    b_ab_ = np.asarray(b_abnor, np.float32)
    W_st = Wnor_p + Wab_p
    b_st = b_nor_ + b_ab_
    wbk = np.zeros((P, 6 * P + 3), np.float32)
    wbk[:, 0:P] = np.asarray(W_in, np.float32)[:P, :]
    wbk[:, P:2 * P] = np.asarray(W_in, np.float32)[P:, :]
    wbk[:, 2 * P:3 * P] = 0.5 * W_st                       # r = 0.5(xn+xa)
    wbk[:, 3 * P:4 * P] = 0.5 * (Wnor_p - Wab_p)           # ddh = 0.5(xn-xa)
    wbk[:, 4 * P:5 * P] = W_st @ np.asarray(W_att, np.float32)
    wbk[:, 5 * P:6 * P] = np.asarray(W_upd, np.float32)
    wbk[:, 6 * P:6 * P + 2] = np.asarray(W_cls, np.float32)
    wbk[:, 6 * P + 2] = np.asarray(v_att, np.float32).reshape(-1) * 0.5

    bias = np.zeros((P, 6), np.float32)
    bias[:, 0] = np.asarray(b_in, np.float32)
    bias[:, 1] = 0.5 * b_st
    bias[:, 2] = 0.5 * (b_nor_ - b_ab_)
    bias[:, 3] = (b_st @ np.asarray(W_att, np.float32)
                  + np.asarray(b_att, np.float32))
    bias[:, 4] = np.asarray(b_upd, np.float32)
    bias[:OUT_DIM, 5] = np.asarray(b_cls, np.float32)

    shared = {
        "wb": wbk.astype(bf),
        "bias": bias,
        "iota_tk": meta["iota_tk"].astype(bf),
        "b_ddh_r": (0.5 * (b_nor_ - b_ab_)).reshape(1, P).astype(bf),
    }
    x_t = meta["x_t"]
    in_maps = []
    for c in range(N_CORES):
        xc = np.concatenate([x_t[:P, c * NSH:(c + 1) * NSH],
                             x_t[P:, c * NSH:(c + 1) * NSH]], axis=1)
        in_maps.append({
            **shared,
            "xq": np.ascontiguousarray(xc).astype(bf),
            "deg_ct": meta["deg_ct"][c],
            "deg_r": meta["deg_r"][c],
            "idx": meta["idx_all"][c],
            "cmw": meta["cm_all"][c].astype(bf),
        })

    results = _run_spmd_presharded(nc, in_maps)
    out_full = np.empty((meta["NP"], OUT_DIM), np.float32)
    for c in range(N_CORES):
        out_full[c * NSH:(c + 1) * NSH] = results[c]["outp"].T
    return np.ascontiguousarray(out_full[:n])


# revision 32
# speedup vs baseline: 1.5662x; 1.0219x over previous
"""CGNN message-passing kernel for Trainium2, 8 NeuronCores.

Strategy (v3):
  - Algebraic reduction: the attention gate depends only on the source node,
    so the edge computation collapses to aggr = dinv (x) (A @ m') with
    m'[j] = dinv_j*(alpha_j*xn_j + (1-alpha_j)*xa_j) a per-node table.
  - Contiguous node sharding: core c owns nodes [c*NSH, (c+1)*NSH) for both
    phases; self-loop terms are added from SBUF (no gathering of self rows).
  - Node phase is fully feature-major and slab-batched: the alpha logit is a
    [1, n] row matmul, alpha*dinv and dinv broadcasts are outer-product
    matmuls, the message mix is three slab-wide DVE ops; only the final
    node-major table rows need a PE transpose per tile. Sigmoid is realized
    as 0.5*tanh(q/2)+0.5 (v_att pre-halved) so Lrelu/Tanh/Identity share one
    activation table.
  - Edge phase: edges sorted by (4-tile quad, table half, target); one SWDGE
    dma_gather per (quad, half) with exact max-count index streams. The
    scatter-add is PSUM-accumulated matmuls over a narrow one-hot: chunks of
    128 target-sorted edges span only ~8 targets, so each matmul writes a
    32-column window of the packed [128, 512] quad PSUM. Window offsets are
    compile-time (union over cores, computed per run); per-core variability
    lives in the cm data (sentinels mask pad slots). The one-hot is built in
    [p, t, k] layout against a materialized iota so all DVE operands are
    packed 2-byte -> 2x DVE rate.
"""
import numpy as np

N_CORES = 8
P = 128
IN_DIM = 256
HID = 128
HALF = 64
OUT_DIM = 2
LRELU_SLOPE = 0.01
SLAB = 4                  # node-phase tiles per slab (<=512 cols)
G = 4                     # edge-phase tiles per quad
W = 32                    # one-hot window width
SENT = 200.0              # one-hot sentinel (never matches a window col)


def _wrap16(flat):
    """[n] int16 -> [128, ceil(n/16)] in the SWDGE 16-wrap + 8x replicated
    layout (flat position i lands at row i%16, col i//16)."""
    n = len(flat)
    cols = -(-n // 16)
    pad = np.zeros(cols * 16, np.int16)
    pad[:n] = flat
    blk = pad.reshape(cols, 16).T          # [16, cols]
    return np.tile(blk, (8, 1))            # [128, cols]


def _host_plan(x, edge_index):
    n = x.shape[0]
    NP = ((n + 1023) // 1024) * 1024       # 50176
    NSH = NP // N_CORES                    # 6272
    T_C = NSH // P                         # 49
    _min_a = -(-max(0, NSH - 4095) // P)   # int16: 8*SPB <= 32767
    SPL_T = max(SLAB, -(-_min_a // SLAB) * SLAB)   # 20 tiles -> table A
    SPA = SPL_T * P                        # 3072 rows/core in table A
    SPB = NSH - SPA                        # 3200 rows/core in table B
    NQ = -(-T_C // G)                      # 13 quads (last has 1 tile)

    ei = np.asarray(edge_index)
    row = ei[0].astype(np.int64)
    col = ei[1].astype(np.int64)

    deg = np.bincount(col, minlength=NP).astype(np.float32)
    deg[:n] += 1.0                         # self loops
    deg[n:] = 1.0                          # pad nodes stay finite

    core_e = col // NSH
    r_e = row % NSH
    half_e = (r_e >= SPA).astype(np.int64)
    src_c = row // NSH
    src_remap = np.where(half_e == 0, src_c * SPA + r_e,
                         src_c * SPB + (r_e - SPA))
    lt_e = (col % NSH) // P                # local tile 0..48
    q_e = lt_e // G

    # per (core, quad, half) edge counts -> static stream sizes
    cnt = np.zeros((N_CORES, NQ, 2), np.int64)
    np.add.at(cnt, (core_e, q_e, half_e), 1)
    nmax = cnt.max(axis=0)                 # [NQ, 2]
    nidx_stat = ((nmax + 127) // 128) * 128  # num_idxs per gather
    CQH = -(-nidx_stat // P)               # gather chunks
    idx_cols = nidx_stat // 16
    idx_off = np.zeros((NQ, 2), np.int64)
    io = 0
    for q in range(NQ):
        for h in range(2):
            idx_off[q, h] = io
            io += idx_cols[q, h]
    IDXC = int(io)
    CMAXG = int(CQH.max())

    order = np.lexsort((col, half_e, q_e, core_e))
    rms = src_remap[order]
    cs = col[order]
    bounds = np.concatenate([[0], np.cumsum(cnt.reshape(-1))])

    # per-core streams: sources, tile-in-quad, local col
    streams = {}
    for c in range(N_CORES):
        for q in range(NQ):
            for h in range(2):
                bi = (c * NQ + q) * 2 + h
                b0, b1 = bounds[bi], bounds[bi + 1]
                streams[(c, q, h)] = (rms[b0:b1],
                                      (cs[b0:b1] % NSH) // P - q * G,
                                      (cs[b0:b1] % P))

    # jobs: per (q, h): [(chunk, tile-in-quad, window offset)], windows from
    # the union of target cols over cores (exact, computed per run)
    jobs = {}
    slot_off = np.zeros((NQ, 2), np.int64)
    so = 0
    KSLOT = 0
    for q in range(NQ):
        for h in range(2):
            slot_off[q, h] = so
            jl = []
            ucols = {}
            for c in range(N_CORES):
                _, tbs, lcs = streams[(c, q, h)]
                pos = np.arange(len(tbs))
                key = (pos // P) * G + tbs
                for kk in np.unique(key):
                    m = key == kk
                    s = ucols.setdefault(int(kk), set())
                    s.update(lcs[m].tolist())
            for kk in sorted(ucols):
                ch, tb = kk // G, kk % G
                colss = sorted(ucols[kk])
                i = 0
                while i < len(colss):
                    o = min(colss[i], P - W)
                    j = i
                    while j < len(colss) and colss[j] < o + W:
                        j += 1
                    jl.append((ch, tb, o))
                    i = j
            jobs[(q, h)] = jl
            so += len(jl)
            KSLOT = max(KSLOT, len(jl))
    NSLOT = int(so)

    idx_all = np.zeros((N_CORES, P, IDXC), np.int16)
    cm_all = np.full((N_CORES, P, NSLOT), SENT, np.float32)
    for c in range(N_CORES):
        for q in range(NQ):
            for h in range(2):
                srcs, tbs, lcs = streams[(c, q, h)]
                nqh = int(nidx_stat[q, h])
                stream = np.zeros(nqh, np.int64)
                stream[:len(srcs)] = srcs
                idx_all[c, :, idx_off[q, h]:idx_off[q, h] + idx_cols[q, h]] = \
                    _wrap16(stream.astype(np.int16))
                pos = np.arange(len(tbs))
                assigned = np.zeros(len(tbs), bool)
                for sl, (ch, tb, o) in enumerate(jobs[(q, h)]):
                    m = ((pos // P == ch) & (tbs == tb) & (lcs >= o)
                         & (lcs < o + W) & ~assigned)
                    if m.any():
                        assigned |= m
                        cm_all[c, pos[m] % P, slot_off[q, h] + sl] = lcs[m] - o

    x_t = np.zeros((IN_DIM, NP), np.float32)
    x_t[:, :n] = np.asarray(x, np.float32).T

    deg_ct = deg.reshape(N_CORES, T_C, P).transpose(0, 2, 1)  # [c, 128, T_C]
    deg_r = deg.reshape(N_CORES, 1, NSH)

    # iota for the windowed one-hot: value t at (p, t*KSLOT + k)
    iota_tk = np.tile(np.repeat(np.arange(W, dtype=np.float32), KSLOT)[None, :],
                      (P, 1))

    return dict(NP=NP, NSH=NSH, T_C=T_C, SPL_T=SPL_T, SPA=SPA, SPB=SPB,
                NQ=NQ,
                CQH=CQH, nidx_stat=nidx_stat, idx_cols=idx_cols,
                idx_off=idx_off, slot_off=slot_off, jobs=jobs,
                IDXC=IDXC, NSLOT=NSLOT, CMAXG=CMAXG, KSLOT=KSLOT,
                idx_all=idx_all, cm_all=cm_all, x_t=x_t, deg_ct=deg_ct,
                deg_r=deg_r, iota_tk=iota_tk)


def _build_program(meta, with_collective=True, act_lrelu=True):
    import concourse.bass as bass
    import concourse.bacc as bacc
    import concourse.mybir as mybir
    import concourse.tile as tile
    from concourse.masks import make_identity

    f32 = mybir.dt.float32
    bf16 = mybir.dt.bfloat16
    i16 = mybir.dt.int16
    AF = mybir.ActivationFunctionType
    Alu = mybir.AluOpType

    NSH, T_C, NP = meta["NSH"], meta["T_C"], meta["NP"]
    SPL_T, SPA, SPB = meta["SPL_T"], meta["SPA"], meta["SPB"]
    NQ = meta["NQ"]
    CQH = meta["CQH"]
    nidx_stat = meta["nidx_stat"]
    idx_cols = meta["idx_cols"]
    idx_off = meta["idx_off"]
    slot_off = meta["slot_off"]
    jobs = meta["jobs"]
    IDXC, NSLOT, CMAXG, KSLOT = (meta["IDXC"], meta["NSLOT"], meta["CMAXG"],
                                 meta["KSLOT"])

    nc = bacc.Bacc("TRN2", target_bir_lowering=False, debug=False,
                   num_swdge_queues=4)
    table_a = nc.dram_tensor("cc_table_a", [N_CORES * SPA, HID], bf16,
                             addr_space="Shared")
    table_b = nc.dram_tensor("cc_table_b", [N_CORES * SPB, HID], bf16,
                             addr_space="Shared")

    d_xq = nc.dram_tensor("xq", [P, 2 * NSH], bf16, kind="ExternalInput")
    d_wb = nc.dram_tensor("wb", [P, 6 * P + 3], bf16, kind="ExternalInput")
    d_bias = nc.dram_tensor("bias", [P, 6], f32, kind="ExternalInput")
    d_degct = nc.dram_tensor("deg_ct", [P, T_C], f32, kind="ExternalInput")
    d_degr = nc.dram_tensor("deg_r", [1, NSH], f32, kind="ExternalInput")
    d_idx = nc.dram_tensor("idx", [P, IDXC], i16, kind="ExternalInput")
    d_cmw = nc.dram_tensor("cmw", [P, NSLOT], bf16, kind="ExternalInput")
    d_iotk = nc.dram_tensor("iota_tk", [P, W * KSLOT], bf16,
                            kind="ExternalInput")
    d_bddh = nc.dram_tensor("b_ddh_r", [1, P], bf16, kind="ExternalInput")
    d_out = nc.dram_tensor("outp", [OUT_DIM, NSH], f32, kind="ExternalOutput")

    WA, WB, WNOR, WAB, WATT, WUPD, WCLS, VH = (0, P, 2 * P, 3 * P, 4 * P,
                                               5 * P, 6 * P, 6 * P + 2)

    def emit_lrelu(out_ap, psum_ap, bias_ap, tmp_pool, nncols):
        if act_lrelu:
            nc.scalar.activation(out_ap, psum_ap, AF.Lrelu, bias=bias_ap,
                                 alpha=LRELU_SLOPE)
        else:
            nc.scalar.activation(out_ap, psum_ap, AF.Identity, bias=bias_ap)
            tl = tmp_pool.tile([P, 512], bf16, tag="lrtmp", name="lrtmp")
            nc.vector.tensor_scalar(out=tl[:, :nncols], in0=out_ap,
                                    scalar1=LRELU_SLOPE, scalar2=None,
                                    op0=Alu.mult)
            nc.vector.tensor_tensor(out=out_ap, in0=out_ap,
                                    in1=tl[:, :nncols], op=Alu.max)

    with tile.TileContext(nc) as tc:
        with (
            tc.tile_pool(name="const", bufs=1) as cpool,
            tc.tile_pool(name="sbuf", bufs=2) as pool,
            tc.tile_pool(name="sbe", bufs=3) as poole,
            tc.tile_pool(name="dram", bufs=1, space="DRAM") as dpool,
        ):
            # ---------- constants ----------
            wb = cpool.tile([P, 6 * P + 3], bf16)
            nc.sync.dma_start(wb[:], d_wb[:])
            bias = cpool.tile([P, 6], f32)
            nc.sync.dma_start(bias[:], d_bias[:])
            iotk = cpool.tile([P, W * KSLOT], bf16)
            idx_sb = cpool.tile([P, IDXC], i16)
            cmw_sb = cpool.tile([P, NSLOT], bf16)

            identb = cpool.tile([P, P], bf16)
            make_identity(nc, identb[:])
            identb4 = cpool.tile([P, G * P], bf16)
            for j in range(G):
                nc.vector.tensor_scalar(out=identb4[:, j * P:(j + 1) * P],
                                        in0=identb[:], scalar1=1.0,
                                        scalar2=None, op0=Alu.mult)
            zerosb = cpool.tile([P, P], bf16)
            nc.vector.memset(zerosb[:], 0.0)
            ones1 = cpool.tile([1, P], bf16)
            nc.vector.memset(ones1[:], 1.0)
            onesr = cpool.tile([1, 512], bf16)
            nc.vector.memset(onesr[:], 1.0)
            bddh = cpool.tile([1, P], bf16)
            nc.sync.dma_start(bddh[:], d_bddh[:])

            dct = cpool.tile([P, T_C], f32)
            nc.sync.dma_start(dct[:], d_degct[:])
            nc.scalar.activation(dct[:], dct[:], AF.Sqrt)
            nc.vector.reciprocal(dct[:], dct[:])
            # flat bf16 dinv row [1, NSH]: PE-transpose dct, bounce via DRAM
            dctb = cpool.tile([P, P], bf16)
            nc.vector.memset(dctb[:], 1.0)
            nc.scalar.activation(dctb[:, :T_C], dct[:], AF.Identity)
            dinvr_b = cpool.tile([1, NSH], bf16)
            scr = dpool.tile([P, P], bf16)
            with tc.tile_pool(name="tps", bufs=1, space="PSUM") as tpp:
                ptc = tpp.tile([P, P], bf16)
                nc.tensor.transpose(ptc[:], dctb[:], identb[:])
                dctT = cpool.tile([P, P], bf16)
                nc.scalar.activation(dctT[:], ptc[:], AF.Identity)
                nc.sync.dma_start(scr[:], dctT[:])
                nc.sync.dma_start(
                    dinvr_b[:],
                    scr[:].rearrange("t p -> (t p)")[:NSH].unsqueeze(0))

            # persistent node-phase products (feature-major)
            selfd = cpool.tile([P, NSH], bf16)    # dinv (x) m' (self term)
            dinvbb = cpool.tile([P, NSH], bf16)   # dinv[t] bcast per column

            shard_a = dpool.tile([SPA, HID], bf16)
            shard_b = dpool.tile([SPB, HID], bf16)
            shard3a = shard_a[:].rearrange("(t p) f -> p t f", p=P)
            shard3b = shard_b[:].rearrange("(t p) f -> p t f", p=P)
            # spill for phase-A partial aggregates
            spill = cpool.tile([P, G * P * NQ], bf16)

            # ---------- node phase ----------
            with tc.tile_pool(name="npsum", bufs=4, space="PSUM") as npp, \
                 tc.tile_pool(name="nppal", bufs=2, space="PSUM") as nppal, \
                 tc.tile_pool(name="nptr", bufs=2, space="PSUM") as nptr:
                t0 = 0
                while t0 < T_C:
                    nt = min(SLAB, T_C - t0)
                    nn = nt * P
                    nb = t0 * P
                    pdb = npp.tile([P, 512], f32, tag="mm")
                    nc.tensor.matmul(pdb[:, :nn], ones1[:],
                                     dinvr_b[:, nb:nb + nn], start=True,
                                     stop=True)
                    nc.scalar.activation(dinvbb[:, nb:nb + nn], pdb[:, :nn],
                                         AF.Identity)
                    t0 += nt
                slabs = []
                t0 = 0
                while t0 < T_C:
                    nt = min(SLAB, T_C - t0)
                    slabs.append((t0, nt))
                    t0 += nt
                SPL_SLAB = SPL_T // SLAB
                state = {}

                def stage1(i):
                    t0, nt = slabs[i]
                    nn = nt * P
                    nb = t0 * P
                    xsl = pool.tile([P, 2 * 512], bf16, tag="xsl",
                                    name="xsl")
                    nc.sync.dma_start(
                        xsl[:, :2 * nn].rearrange("p (a n) -> p a n", a=2),
                        d_xq[:].rearrange("p (a n) -> p a n",
                                          a=2)[:, :, nb:nb + nn])
                    ph = npp.tile([P, 512], f32, tag="mm", name="ph")
                    nc.tensor.matmul(ph[:, :nn], wb[:, WA:WA + P],
                                     xsl[:, :nn], start=True, stop=False)
                    nc.tensor.matmul(ph[:, :nn], wb[:, WB:WB + P],
                                     xsl[:, nn:2 * nn],
                                     start=False, stop=True)
                    h = pool.tile([P, 512], bf16, tag="h", name="h")
                    emit_lrelu(h[:, :nn], ph[:, :nn], bias[:, 0:1], pool, nn)
                    pr = npp.tile([P, 512], f32, tag="mm", name="pr")
                    nc.tensor.matmul(pr[:, :nn], wb[:, WNOR:WNOR + P],
                                     h[:, :nn], start=True, stop=True)
                    pd = npp.tile([P, 512], f32, tag="mm", name="pd")
                    nc.tensor.matmul(pd[:, :nn], wb[:, WAB:WAB + P],
                                     h[:, :nn], start=True, stop=False)
                    nc.tensor.matmul(pd[:, :nn], bddh[:], onesr[:, :nn],
                                     start=False, stop=True)
                    patt = npp.tile([P, 512], f32, tag="mm", name="patt")
                    nc.tensor.matmul(patt[:, :nn], wb[:, WATT:WATT + P],
                                     h[:, :nn], start=True, stop=True)
                    rr = pool.tile([P, 512], bf16, tag="rr", name="rr")
                    nc.vector.tensor_scalar(out=rr[:, :nn], in0=pr[:, :nn],
                                            scalar1=bias[:, 1:2],
                                            scalar2=None, op0=Alu.add)
                    hatt = pool.tile([P, 512], bf16, tag="hatt", name="hatt")
                    nc.scalar.activation(hatt[:, :nn], patt[:, :nn], AF.Tanh,
                                         bias=bias[:, 3:4])
                    pal = nppal.tile([1, 512], f32, tag="pal", name="pal")
                    nc.tensor.matmul(pal[:, :nn], wb[:, VH:VH + 1],
                                     hatt[:, :nn], start=True, stop=True)
                    sig = pool.tile([1, 512], bf16, tag="sig", name="sig")
                    nc.scalar.activation(sig[:, :nn], pal[:, :nn], AF.Tanh)
                    state[i] = (rr, pd, sig)

                def stage2(i):
                    t0, nt = slabs[i]
                    nn = nt * P
                    nn2 = (nt // 2) * P if nt > 1 else nn
                    nb = t0 * P
                    rr, pd, sig = state.pop(i)
                    psg = npp.tile([P, 512], f32, tag="mm", name="psg")
                    nc.tensor.matmul(psg[:, :nn], ones1[:], sig[:, :nn],
                                     start=True, stop=True)
                    sgb = pool.tile([P, 512], f32, tag="sgb", name="sgb")
                    nc.scalar.activation(sgb[:, :nn], psg[:, :nn],
                                         AF.Identity)
                    s2 = pool.tile([P, 512], bf16, tag="s2", name="s2")
                    nc.vector.tensor_mul(s2[:, :nn], pd[:, :nn],
                                         sgb[:, :nn])
                    ms = pool.tile([P, 512], bf16, tag="ms", name="ms")
                    nc.vector.tensor_add(ms[:, :nn], rr[:, :nn], s2[:, :nn])
                    mfs = pool.tile([P, 512], bf16, tag="mfs", name="mfs")
                    nc.vector.tensor_mul(mfs[:, :nn], ms[:, :nn],
                                         dinvbb[:, nb:nb + nn])
                    nc.vector.tensor_mul(selfd[:, nb:nb + nn], mfs[:, :nn],
                                         dinvbb[:, nb:nb + nn])
                    ptm = nptr.tile([P, 512], bf16, tag="tr", name="tr")
                    for j in range(nt):
                        nc.tensor.transpose(
                            ptm[:, j * P:(j + 1) * P],
                            mfs[:, j * P:(j + 1) * P], identb[:])
                    mrow = pool.tile([P, 512], bf16, tag="mrow", name="mrow")
                    nc.scalar.activation(mrow[:, :nn2], ptm[:, :nn2],
                                         AF.Identity)
                    if nn > nn2:
                        nc.vector.tensor_scalar(out=mrow[:, nn2:nn],
                                                in0=ptm[:, nn2:nn],
                                                scalar1=1.0, scalar2=None,
                                                op0=Alu.mult)
                    if t0 < SPL_T:
                        nc.sync.dma_start(
                            shard3a[:, t0:t0 + nt, :],
                            mrow[:, :nn].rearrange("p (t f) -> p t f", f=P))
                    else:
                        nc.sync.dma_start(
                            shard3b[:, t0 - SPL_T:t0 - SPL_T + nt, :],
                            mrow[:, :nn].rearrange("p (t f) -> p t f", f=P))

                for i in range(len(slabs)):
                    stage1(i)
                    if i == 1:
                        nc.sync.dma_start(iotk[:], d_iotk[:])
                        nc.sync.dma_start(idx_sb[:], d_idx[:])
                        nc.sync.dma_start(cmw_sb[:], d_cmw[:])
                    if i > 0:
                        stage2(i - 1)
                    if i == SPL_SLAB:
                        if with_collective:
                            nc.gpsimd.collective_compute(
                                "AllGather", mybir.AluOpType.bypass,
                                replica_groups=[list(range(N_CORES))],
                                ins=[shard_a.opt()], outs=[table_a[:]])
                        else:
                            nc.sync.dma_start(table_a[:SPA, :], shard_a[:])
                stage2(len(slabs) - 1)

            # ---------- replicate table (half B) ----------
            if with_collective:
                nc.gpsimd.collective_compute(
                    "AllGather",
                    mybir.AluOpType.bypass,
                    replica_groups=[list(range(N_CORES))],
                    ins=[shard_b.opt()],
                    outs=[table_b[:]],
                )
            else:
                nc.sync.dma_start(table_b[:SPB, :], shard_b[:])

            # ---------- edge phase ----------
            with tc.tile_pool(name="epsum", bufs=4, space="PSUM") as epp, \
                 tc.tile_pool(name="eps2", bufs=2, space="PSUM") as epp2:

                def emit_gather_onehot(q, hh, tbl):
                    C = int(CQH[q, hh])
                    nix = int(nidx_stat[q, hh])
                    nj = len(jobs[(q, hh)])
                    g = poole.tile([P, CMAXG * HID], bf16, tag=f"g{hh}",
                                   name=f"g{hh}")
                    soh = poole.tile([P, W * KSLOT], bf16, tag=f"soh{hh}",
                                     name=f"soh{hh}")
                    # SWDGE ring caps one gather at 1024 descriptors
                    c0 = 0
                    while c0 < C:
                        cs_ = min(8, C - c0)
                        ni = cs_ * P
                        nc.gpsimd.dma_gather(
                            out_ap=g[:, c0 * HID:(c0 + cs_) * HID]
                            .rearrange("p (c f) -> p c f", f=HID),
                            in_ap=tbl[:, :],
                            idxs_ap=idx_sb[:, idx_off[q, hh] + c0 * 8:
                                           idx_off[q, hh] + (c0 + cs_) * 8],
                            num_idxs=ni,
                            num_idxs_reg=ni,
                            elem_size=HID,
                            queue_num=(2 * q + hh + c0 // 8) % 4,
                        )
                        c0 += cs_
                    if nj:
                        nc.vector.tensor_tensor(
                            out=soh[:, :W * nj].rearrange(
                                "p (t k) -> p t k", k=nj),
                            in0=iotk[:].rearrange(
                                "p (t k) -> p t k", k=KSLOT)[:, :, :nj],
                            in1=cmw_sb[:, slot_off[q, hh]:
                                       slot_off[q, hh] + nj]
                            .unsqueeze(1).to_broadcast([P, W, nj]),
                            op=Alu.is_equal)
                    return g, soh

                def emit_jobs(q, hh, g, soh, pagg, nn):
                    nj = len(jobs[(q, hh)])
                    nc.tensor.matmul(pagg[:, :nn], zerosb[:],
                                     identb4[:, :nn], start=True,
                                     stop=(nj == 0))
                    if nj == 0:
                        return
                    soh3 = soh[:, :W * nj].rearrange("p (t k) -> p t k", k=nj)
                    for sl, (ch, tb, o) in enumerate(jobs[(q, hh)]):
                        ob = tb * P + o
                        nc.tensor.matmul(
                            pagg[:, ob:ob + W],
                            g[:, ch * HID:(ch + 1) * HID],
                            soh3[:, :, sl], start=False, stop=False)
                    nc.tensor.matmul(pagg[:, :nn], zerosb[:],
                                     identb4[:, :nn], start=False, stop=True)

                # phase A: gather from table_a, spill partial aggregates
                for q in range(NQ):
                    ntile = min(G, T_C - q * G)
                    nn = ntile * P
                    qb = q * G * P
                    g, soh = emit_gather_onehot(q, 0, table_a)
                    pagg = epp.tile([P, G * P], f32, tag="pagg")
                    emit_jobs(q, 0, g, soh, pagg, nn)
                    nc.scalar.activation(spill[:, qb:qb + nn], pagg[:, :nn],
                                         AF.Identity)

                # phase B: gather from table_b, combine + output layers
                for q in range(NQ):
                    ntile = min(G, T_C - q * G)
                    nn = ntile * P
                    qb = q * G * P
                    g, soh = emit_gather_onehot(q, 1, table_b)
                    pagg = epp.tile([P, G * P], f32, tag="pagg")
                    emit_jobs(q, 1, g, soh, pagg, nn)
                    paggb = pool.tile([P, G * P], bf16, tag="paggb")
                    nc.scalar.activation(paggb[:, :nn], pagg[:, :nn],
                                         AF.Identity)
                    u1 = pool.tile([P, G * P], bf16, tag="u1")
                    nc.vector.tensor_add(u1[:, :nn], paggb[:, :nn],
                                         spill[:, qb:qb + nn])
                    m2a = pool.tile([P, G * P], bf16, tag="m2a")
                    nc.vector.tensor_mul(m2a[:, :nn], u1[:, :nn],
                                         dinvbb[:, qb:qb + nn])
                    m2b = pool.tile([P, G * P], bf16, tag="m2b")
                    nc.vector.tensor_add(m2b[:, :nn], m2a[:, :nn],
                                         selfd[:, qb:qb + nn])
                    pu = epp2.tile([P, G * P], f32, tag="pu")
                    nc.tensor.matmul(pu[:, :nn], wb[:, WUPD:WUPD + P],
                                     m2b[:, :nn], start=True, stop=True)
                    lu = pool.tile([P, G * P], bf16, tag="lu")
                    emit_lrelu(lu[:, :nn], pu[:, :nn], bias[:, 4:5], pool, nn)
                    po = epp2.tile([OUT_DIM, G * P], f32, tag="po")
                    nc.tensor.matmul(po[:, :nn], wb[:, WCLS:WCLS + OUT_DIM],
                                     lu[:, :nn], start=True, stop=True)
                    ot = pool.tile([OUT_DIM, G * P], f32, tag="ot")
                    nc.scalar.activation(ot[:, :nn], po[:, :nn], AF.Identity,
                                         bias=bias[0:OUT_DIM, 5:6])
                    nc.sync.dma_start(d_out[:, qb:qb + nn], ot[:, :nn])

    nc.compile()
    return nc


def _run_spmd_presharded(nc, in_maps, n_cores=N_CORES):
    """Run a compiled Bass program on n_cores via PJRT with host-side
    pre-sharded inputs (avoids XLA reshard programs on big arrays)."""
    import jax
    import concourse.mybir as mybir
    from concourse import bass2jax
    from jax.sharding import Mesh, PartitionSpec, NamedSharding
    from jax.experimental.shard_map import shard_map

    bass2jax.install_neuronx_cc_hook()
    partition_name = nc.partition_id_tensor.name if nc.partition_id_tensor else None
    in_names, out_names, out_avals, zero_outs = [], [], [], []
    for alloc in nc.m.functions[0].allocations:
        if not isinstance(alloc, mybir.MemoryLocationSet):
            continue
        name = alloc.memorylocations[0].name
        if alloc.kind == "ExternalInput":
            if name != partition_name:
                in_names.append(name)
        elif alloc.kind == "ExternalOutput":
            out_names.append(name)
            shape = tuple(alloc.tensor_shape)
            dtype = mybir.dt.np(alloc.dtype)
            out_avals.append(jax.core.ShapedArray(shape, dtype))
            zero_outs.append(np.zeros(shape, dtype))
    n_params = len(in_names)
    in_names_all = list(in_names) + out_names
    if partition_name is not None:
        in_names_all.append(partition_name)

    def _body(*args):
        operands = list(args)
        if partition_name is not None:
            operands.append(bass2jax.partition_id_tensor())
        outs = bass2jax._bass_exec_p.bind(
            *operands,
            out_avals=tuple(out_avals),
            in_names=tuple(in_names_all),
            out_names=tuple(out_names),
            lowering_input_output_aliases=(),
            sim_require_finite=True,
            sim_require_nnan=True,
            nc=nc,
        )
        return tuple(outs)

    devices = jax.devices()[:n_cores]
    mesh = Mesh(np.asarray(devices), ("core",))
    spec = PartitionSpec("core")
    n_outs = len(out_avals)
    sharded = jax.jit(
        shard_map(_body, mesh=mesh, in_specs=(spec,) * (n_params + n_outs),
                  out_specs=(spec,) * n_outs, check_rep=False),
        keep_unused=True,
    )
    sh = NamedSharding(mesh, spec)

    def put(per_core_arrays):
        a0 = np.asarray(per_core_arrays[0])
        gshape = (n_cores * a0.shape[0],) + a0.shape[1:]
        shards = [jax.device_put(np.ascontiguousarray(per_core_arrays[c]),
                                 devices[c]) for c in range(n_cores)]
        return jax.make_array_from_single_device_arrays(gshape, sh, shards)

    args = [put([m[name] for m in in_maps]) for name in in_names]
    args += [put([z] * n_cores) for z in zero_outs]
    out_arrs = sharded(*args)
    jax.block_until_ready(out_arrs)
    return [
        {name: np.asarray(out_arrs[i]).reshape(n_cores, *out_avals[i].shape)[c]
         for i, name in enumerate(out_names)}
        for c in range(n_cores)
    ]


def kernel(x, edge_index, W_in, b_in, W_nor, b_nor, W_abnor, b_abnor,
           W_att, b_att, v_att, W_upd, b_upd, W_cls, b_cls):
    import ml_dtypes
    bf = ml_dtypes.bfloat16

    x = np.asarray(x, np.float32)
    n = x.shape[0]
    meta = _host_plan(x, edge_index)
    NSH = meta["NSH"]
    nc = _build_program(meta, with_collective=True)

    Wnor_p = np.zeros((P, P), np.float32)
    Wnor_p[:HALF] = np.asarray(W_nor, np.float32)
    Wab_p = np.zeros((P, P), np.float32)
    Wab_p[HALF:] = np.asarray(W_abnor, np.float32)
    b_nor_ = np.asarray(b_nor, np.float32)
    b_ab_ = np.asarray(b_abnor, np.float32)
    W_st = Wnor_p + Wab_p
    b_st = b_nor_ + b_ab_
    wbk = np.zeros((P, 6 * P + 3), np.float32)
    wbk[:, 0:P] = np.asarray(W_in, np.float32)[:P, :]
    wbk[:, P:2 * P] = np.asarray(W_in, np.float32)[P:, :]
    wbk[:, 2 * P:3 * P] = 0.5 * W_st                       # r = 0.5(xn+xa)
    wbk[:, 3 * P:4 * P] = 0.5 * (Wnor_p - Wab_p)           # ddh = 0.5(xn-xa)
    wbk[:, 4 * P:5 * P] = W_st @ np.asarray(W_att, np.float32)
    wbk[:, 5 * P:6 * P] = np.asarray(W_upd, np.float32)
    wbk[:, 6 * P:6 * P + 2] = np.asarray(W_cls, np.float32)
    wbk[:, 6 * P + 2] = np.asarray(v_att, np.float32).reshape(-1) * 0.5

    bias = np.zeros((P, 6), np.float32)
    bias[:, 0] = np.asarray(b_in, np.float32)
    bias[:, 1] = 0.5 * b_st
    bias[:, 2] = 0.5 * (b_nor_ - b_ab_)
    bias[:, 3] = (b_st @ np.asarray(W_att, np.float32)
                  + np.asarray(b_att, np.float32))
    bias[:, 4] = np.asarray(b_upd, np.float32)
    bias[:OUT_DIM, 5] = np.asarray(b_cls, np.float32)

    shared = {
        "wb": wbk.astype(bf),
        "bias": bias,
        "iota_tk": meta["iota_tk"].astype(bf),
        "b_ddh_r": (0.5 * (b_nor_ - b_ab_)).reshape(1, P).astype(bf),
    }
    x_t = meta["x_t"]
    in_maps = []
    for c in range(N_CORES):
        xc = np.concatenate([x_t[:P, c * NSH:(c + 1) * NSH],
                             x_t[P:, c * NSH:(c + 1) * NSH]], axis=1)
        in_maps.append({
            **shared,
            "xq": np.ascontiguousarray(xc).astype(bf),
            "deg_ct": meta["deg_ct"][c],
            "deg_r": meta["deg_r"][c],
            "idx": meta["idx_all"][c],
            "cmw": meta["cm_all"][c].astype(bf),
        })

    results = _run_spmd_presharded(nc, in_maps)
    out_full = np.empty((meta["NP"], OUT_DIM), np.float32)
    for c in range(N_CORES):
        out_full[c * NSH:(c + 1) * NSH] = results[c]["outp"].T
    return np.ascontiguousarray(out_full[:n])
